# revision 29
# baseline (speedup 1.0000x reference)
"""Trainium2 Bass kernel for linear (taylor/sparse) attention.

Reference computation (per batch b, with xf = x.reshape(b, C, N)):
    Q = Wq@xf + bq            [Cqk, N]
    K = Wk@xf + bk            [Cqk, N]
    V = Wv@xf + bv            [C, N]
    Qh = Q / ||Q||_2 (per position, channel dim)
    Kh = K / ||K||_2
    tailor[n]   = 1 / (N + Qh[:,n] . (sum_n Kh + eps))
    matrix      = Kh @ V^T    [Cqk, C]
    out[:, n]   = gamma * tailor[n] * (sum_n V + matrix^T @ Qh[:,n])

Distribution: 8 cores = 4 batches x 2 halves of N (seq parallel), with a
pairwise AllReduce of the tiny factor.

v3 layout (per core, NSH=8192 positions = 64 tiles of 128):
  A dummy 256B collective fires at t=0 to absorb the CC engine's startup
  and the cross-core launch skew, so the real factor AllReduce later pays
  only the mesh transfer.
  Phase A per tile: fused projection [Qraw|Kraw|V'|2bq.Qraw|2bk.Kraw]
  (322 cols, V' = gamma*Wv@x) into a 6-deep PSUM rotation, Q|K|V copied to
  a persistent SBUF ring (scalar), sums-of-squares accumulated from the
  bf16 ring on DVE (bias-free norms: ||K+bk||^2 = ssk_raw + 2bk.Kraw +
  ||bk||^2, cross terms as extra projection columns), and per 8-tile group
  the K-norm reciprocals scale the kh ring so the factor
      psf[34, 257] = [r*Kraw | r | 1]^T @ [V' | 1]
  accumulates bias-free (bk folds in post-collective as a rank-1 PE
  update: mx = sel^T.facg + bk (x) facg_row32 + N*bv' row).  GpSimd stays
  nearly idle in phase A so the AllReduce dispatches the moment the factor
  is ready; phase B (c-major Q+bq via matmul with per-partition bias, the
  ||Q+bq|| row via sqrt + PE transpose) hides the collective latency.
  Phase 2 per 4 tiles: denominator-column matmuls into a shared PSUM bank
  + one batched reciprocal, then [128, 256] matmuls into a 7-deep PSUM
  rotation, scaled on alternating Scalar/Vector into bf16, DMA'd out 8
  tiles per descriptor.
Output is n-major bf16 [N_shard, C]; the host transposes/casts back.
"""

import ml_dtypes
import numpy as np
from contextlib import ExitStack

import concourse.bass as bass
import concourse.bacc as bacc
import concourse.tile as tile
from concourse import mybir
from concourse import bass_utils
from concourse.masks import make_identity

F32 = mybir.dt.float32
BF16 = mybir.dt.bfloat16
ALU = mybir.AluOpType
ACTF = mybir.ActivationFunctionType

B, C, HH, WW = 4, 256, 128, 128
N = HH * WW            # 16384 positions per batch
NSH = N // 2           # 8192 positions per core
CQK = 32
PW = 2 * CQK + C + 2   # 322: [Q | K | V | qcross | kcross] fused projection
KVW = 2 * CQK + C + 1  # 321: kvring slot = [Q | K | V | ones]
FD = C + 1             # 257: factor free width ([V | 1])
NT512 = NSH // 512     # 16
NT128 = NSH // 128     # 64
GRP = 8                # tiles per K-norm batching group
EPS = 1e-6
RG = [[0, 1], [2, 3], [4, 5], [6, 7]]

_CACHE = {}


def _build():
    nc = bacc.Bacc("TRN2", target_bir_lowering=False, debug=False, num_devices=8)

    xs = nc.dram_tensor("xs", [C, NSH], BF16, kind="ExternalInput").ap()
    wcat = nc.dram_tensor("wcat", [C, PW], BF16, kind="ExternalInput").ap()
    bq_in = nc.dram_tensor("bq", [CQK, 1], F32, kind="ExternalInput").ap()
    normc = nc.dram_tensor("normc", [2], F32, kind="ExternalInput").ap()
    bvg = nc.dram_tensor("bvg", [C], F32, kind="ExternalInput").ap()
    mxsel = nc.dram_tensor("mxsel", [CQK + 2, CQK + 1], F32, kind="ExternalInput").ap()
    mxrows = nc.dram_tensor("mxrows", [2, CQK + 1], F32, kind="ExternalInput").ap()
    nbvrow = nc.dram_tensor("nbvrow", [1, FD], F32, kind="ExternalInput").ap()
    out = nc.dram_tensor("out", [NSH, C], BF16, kind="ExternalOutput").ap()

    with tile.TileContext(nc) as tc, ExitStack() as ctx:
        _body(ctx, tc, nc, xs, wcat, bq_in, normc, bvg, mxsel, mxrows, nbvrow, out)

    nc.compile()
    return nc


def _body(ctx, tc, nc, xs, wcat, bq_in, normc, bvg, mxsel, mxrows, nbvrow, out):
    singles = ctx.enter_context(tc.tile_pool(name="singles", bufs=1))
    xpool = ctx.enter_context(tc.tile_pool(name="x", bufs=NT512))
    smalls = ctx.enter_context(tc.tile_pool(name="smalls", bufs=4))
    scpool = ctx.enter_context(tc.tile_pool(name="scratch", bufs=4))
    outpool = ctx.enter_context(tc.tile_pool(name="outp", bufs=2))
    dram = ctx.enter_context(tc.tile_pool(name="dram", bufs=1, space="DRAM"))

    # ---- warm-up collective: absorbs CC startup + cross-core launch skew
    # so the real factor AllReduce later only pays the mesh transfer ----
    warm_in = dram.tile([1, 64], F32)
    warm_out = dram.tile([2, 64], F32)
    nc.gpsimd.collective_compute(
        "AllGather", ALU.bypass, replica_groups=RG,
        ins=[warm_in.opt()], outs=[warm_out.opt()],
    )
    warm_in2 = dram.tile([1, 64], F32)
    warm_out2 = dram.tile([2, 64], F32)

    # ---- one-time setup (small uploads on gpsimd; it idles afterwards so
    # the factor collective dispatches without queueing) ----
    wcat_sb = singles.tile([128, 2, PW], BF16)
    nc.sync.dma_start(wcat_sb[:], wcat.rearrange("(cb cp) w -> cp cb w", cb=2))
    bq_col = singles.tile([CQK, 1], F32)
    nc.gpsimd.dma_start(bq_col[:], bq_in)
    normc_rep = singles.tile([128, 2], F32)
    nc.gpsimd.dma_start(
        normc_rep[:], normc.unsqueeze(0).partition_broadcast(128).squeeze(1)
    )
    bv_rep = singles.tile([CQK, C], F32)
    nc.gpsimd.dma_start(
        bv_rep[:], bvg.unsqueeze(0).partition_broadcast(CQK).squeeze(1)
    )
    mxsel_sb = singles.tile([CQK + 2, CQK + 1], F32)
    nc.gpsimd.dma_start(mxsel_sb[:], mxsel)
    # rank-1 fold operands must sit at base partition 32 (matmul base
    # partitions are restricted to 0/32/64 and must match between operands)
    bkrow_sb = singles.tile([CQK + 1, CQK + 1], F32)   # row 32: [bk | 0]
    nc.gpsimd.dma_start(bkrow_sb[CQK : CQK + 1, :], mxrows[0:1, :])
    erow_sb = singles.tile([CQK + 1, CQK + 1], F32)    # row 32: [0..0 | 1]
    nc.gpsimd.dma_start(erow_sb[CQK : CQK + 1, :], mxrows[1:2, :])
    nbvrow_sb = singles.tile([CQK + 1, FD], F32)       # row 32: [N*g*bv | 0]
    nc.gpsimd.dma_start(nbvrow_sb[CQK : CQK + 1, :], nbvrow)
    ident = singles.tile([128, 128], F32)
    make_identity(nc, ident[:])

    # persistent rings: ones columns preset once
    kvring = singles.tile([128, 16, KVW], BF16)      # [Q | K | V | ones]
    nc.vector.memset(kvring[:, :, KVW - 1 : KVW], 1.0)
    khring = singles.tile([128, GRP, CQK + 2], BF16)  # [r*K | r | ones]
    nc.vector.memset(khring[:, :, CQK + 1 : CQK + 2], 1.0)

    qx = singles.tile([CQK + 1, NSH], BF16)          # c-major Q+bq rows + ||Q|| row
    ssq_stack = singles.tile([128, NT128], F32)      # sum(Qraw^2), col t
    ssk_stack = singles.tile([128, NT128], F32)      # sum(Kraw^2), col t
    crossqk = singles.tile([128, 2, NT128], F32)     # [2*bq.Qraw ; 2*bk.Kraw]
    cc_in = dram.tile([CQK + 2, FD], F32)
    cc_red = dram.tile([CQK + 2, FD], F32)

    xt_tiles = [None] * NT512

    with tc.tile_pool(name="ps_kqv", bufs=7, space="PSUM") as ps_kqv, tc.tile_pool(
        name="ps_f", bufs=1, space="PSUM"
    ) as ps_f:
        psf = ps_f.tile([CQK + 2, FD], F32)          # factor accumulator
        pending_tail = None

        def emit_tail(g0):
            nsum = smalls.tile([128, GRP], F32)
            nc.gpsimd.tensor_tensor(
                nsum[:], ssk_stack[:, g0 : g0 + GRP], crossqk[:, 1, g0 : g0 + GRP],
                ALU.add,
            )
            normk = smalls.tile([128, GRP], F32)
            nc.scalar.activation(
                normk[:], nsum[:], ACTF.Sqrt, bias=normc_rep[:, 1:2], scale=1.0
            )
            rnorm = smalls.tile([128, GRP], F32)
            nc.vector.reciprocal(rnorm[:], normk[:])
            # r column of the kh ring (slot tt%GRP), bf16 for the factor matmul
            nc.gpsimd.tensor_copy(khring[:, 0:GRP, CQK : CQK + 1], rnorm[:])
            for tt in range(g0, g0 + GRP):
                sl = tt % GRP
                nc.gpsimd.tensor_scalar_mul(
                    khring[:, sl, 0:CQK],
                    kvring[:, tt % 16, CQK : 2 * CQK],
                    rnorm[:, sl : sl + 1],
                )
                nc.tensor.matmul(
                    psf[:], khring[:, sl, :], kvring[:, tt % 16, 2 * CQK : KVW],
                    start=(tt == 0), stop=(tt == NT128 - 1),
                )

        # ---- phase A: projections + norm accums + factor ----
        for j in range(NT512):
            if j == NT512 // 2:
                # re-align the cores mid-phase so the factor collective
                # below sees minimal peer skew
                nc.gpsimd.collective_compute(
                    "AllGather", ALU.bypass, replica_groups=RG,
                    ins=[warm_in2.opt()], outs=[warm_out2.opt()],
                )
            xt = xpool.tile([128, 2, 512], BF16)
            nc.sync.dma_start(
                xt[:],
                xs.rearrange("(cb cp) n -> cp cb n", cb=2)[
                    :, :, j * 512 : (j + 1) * 512
                ],
            )
            xt_tiles[j] = xt

            for u in range(4):
                t = j * 4 + u
                if u == 2 and j % 2 == 0 and pending_tail is not None:
                    emit_tail(pending_tail)
                    pending_tail = None
                pskqv = ps_kqv.tile([128, PW], F32)
                for cb in range(2):
                    nc.tensor.matmul(
                        pskqv[:], xt[:, cb, u * 128 : (u + 1) * 128],
                        wcat_sb[:, cb, :],
                        start=(cb == 0), stop=(cb == 1),
                    )
                # Q|K -> ring on vector, V -> ring on scalar (bf16 casts);
                # squares accumulate from the SBUF bf16 copy on gpsimd
                # (engines read only one PSUM input; gpsimd reads none)
                nc.vector.tensor_copy(
                    kvring[:, t % 16, 0 : 2 * CQK], pskqv[:, 0 : 2 * CQK]
                )
                nc.scalar.copy(
                    kvring[:, t % 16, 2 * CQK : 2 * CQK + C],
                    pskqv[:, 2 * CQK : 2 * CQK + C],
                )
                scr_q = scpool.tile([128, CQK], BF16)
                scr_k = scpool.tile([128, CQK], BF16)
                nc.vector.scalar_tensor_tensor(
                    scr_q[:], kvring[:, t % 16, 0:CQK], 1.0,
                    kvring[:, t % 16, 0:CQK],
                    ALU.mult, ALU.mult, accum_out=ssq_stack[:, t : t + 1],
                )
                nc.vector.scalar_tensor_tensor(
                    scr_k[:],
                    kvring[:, t % 16, CQK : 2 * CQK],
                    1.0,
                    kvring[:, t % 16, CQK : 2 * CQK],
                    ALU.mult, ALU.mult, accum_out=ssk_stack[:, t : t + 1],
                )
                nc.vector.tensor_copy(
                    crossqk[:, :, t : t + 1], pskqv[:, PW - 2 : PW]
                )

            if (j + 1) % (GRP // 4) == 0:
                pending_tail = (j + 1) * 4 - GRP
        if pending_tail is not None:
            emit_tail(pending_tail)
            pending_tail = None

        # ---- collective: fire as soon as the factor is done ----
        fac_loc = singles.tile([CQK + 2, FD], F32)
        nc.vector.tensor_copy(fac_loc[:], psf[:])
        nc.sync.dma_start(cc_in[:], fac_loc[:])
        nc.gpsimd.collective_compute(
            "AllReduce", ALU.add, replica_groups=RG,
            ins=[cc_in.opt()], outs=[cc_red.opt()],
        )

    with tc.tile_pool(name="ps_b", bufs=4, space="PSUM") as ps_b:
        # ---- phase B (hides the collective): c-major Q+bq, ||Q+bq|| row ----
        for j in range(NT512):
            psq = ps_b.tile([CQK, 512], F32, tag="shared")
            for cb in range(2):
                nc.tensor.matmul(
                    psq[:], wcat_sb[:, cb, 0:CQK], xt_tiles[j][:, cb, :],
                    start=(cb == 0), stop=(cb == 1),
                )
            if j % 8 < 5:
                nc.scalar.activation(
                    qx[0:CQK, j * 512 : (j + 1) * 512], psq[:],
                    ACTF.Identity, bias=bq_col[:], scale=1.0,
                )
            else:
                nc.vector.tensor_scalar_add(
                    qx[0:CQK, j * 512 : (j + 1) * 512], psq[:], bq_col[:]
                )

        ssq_tot = singles.tile([128, NT128], F32)
        nc.vector.tensor_tensor(ssq_tot[:], ssq_stack[:], crossqk[:, 0, :], ALU.add)
        normq_stack = singles.tile([128, NT128], F32)
        nc.scalar.activation(
            normq_stack[:], ssq_tot[:], ACTF.Sqrt, bias=normc_rep[:, 0:1], scale=1.0
        )
        pst = ps_b.tile([NT128, 128], F32, tag="shared")
        nc.tensor.transpose(pst[:], normq_stack[:], ident[:])
        trT = singles.tile([NT128, 128], BF16)
        nc.vector.tensor_copy(trT[:], pst[:])
        row_scratch = dram.tile([NT128, 128], BF16)
        nc.sync.dma_start(row_scratch[:], trT[:])
        nc.sync.dma_start(
            qx[CQK : CQK + 1, :],
            row_scratch[:].rearrange("a b -> (a b)").unsqueeze(0),
        )

        # ---- post-collective: facg + mx build (PE rank-1 folds) ----
        facg = singles.tile([CQK + 2, FD], F32)
        nc.sync.dma_start(facg[:], cc_red[:])
        ps_mx = ps_b.tile([CQK + 1, FD], F32, tag="shared")
        nc.tensor.matmul(ps_mx[:], mxsel_sb[:], facg[:], start=True, stop=False)
        nc.tensor.matmul(
            ps_mx[:],
            bkrow_sb[CQK : CQK + 1, :],
            facg[CQK : CQK + 1, :],
            start=False,
            stop=False,
        )
        nc.tensor.matmul(
            ps_mx[:],
            erow_sb[CQK : CQK + 1, :],
            nbvrow_sb[CQK : CQK + 1, :],
            start=False,
            stop=True,
        )
        mx = singles.tile([CQK + 1, FD], BF16)
        # rows 0:32: matrix + Ksum (x) bv'  (Ksum = ps_mx col 256)
        nc.vector.scalar_tensor_tensor(
            mx[0:CQK, 0:C], bv_rep[:], ps_mx[0:CQK, C : C + 1], ps_mx[0:CQK, 0:C],
            ALU.mult, ALU.add,
        )
        nc.vector.tensor_scalar_add(
            mx[0:CQK, C : C + 1], ps_mx[0:CQK, C : C + 1], EPS
        )
        nc.vector.tensor_copy(mx[CQK : CQK + 1, :], ps_mx[CQK : CQK + 1, :])

        # ---- phase 2 ----
        with tc.tile_pool(name="ps_p2", bufs=3, space="PSUM") as ps_p2:
            out8 = out.rearrange("(g u p) c -> g p u c", u=GRP, p=128)
            for g8 in range(NT128 // GRP):
                ot = outpool.tile([128, GRP, C], BF16)
                for half in range(2):
                    g4 = g8 * 2 + half
                    psden = ps_p2.tile([128, 4], F32, tag="den", bufs=1)
                    for i in range(4):
                        t = g4 * 4 + i
                        nc.tensor.matmul(
                            psden[:, i : i + 1],
                            qx[:, t * 128 : (t + 1) * 128],
                            mx[:, C : C + 1],
                            start=True, stop=True,
                        )
                    rec4 = smalls.tile([128, 4], F32)
                    nc.vector.reciprocal(rec4[:], psden[:])
                    for i in range(4):
                        t = g4 * 4 + i
                        u = half * 4 + i
                        if i % 2 == 0:
                            ps2 = ps_b.tile([128, C], F32, tag="shared")
                        else:
                            ps2 = ps_p2.tile([128, C], F32, tag="p2")
                        nc.tensor.matmul(
                            ps2[:], qx[:, t * 128 : (t + 1) * 128], mx[:, 0:C],
                            start=True, stop=True,
                        )
                        # scale split by columns across both engines
                        nc.vector.tensor_scalar_mul(
                            ot[:, u, 0 : C // 2], ps2[:, 0 : C // 2],
                            rec4[:, i : i + 1],
                        )
                        nc.scalar.mul(
                            ot[:, u, C // 2 : C], ps2[:, C // 2 : C],
                            rec4[:, i : i + 1],
                        )
                nc.sync.dma_start(out8[g8], ot[:])


def _get_nc():
    if "nc" not in _CACHE:
        _CACHE["nc"] = _build()
    return _CACHE["nc"]


def _prep_in_maps(x, Wq, bq, Wk, bk, Wv, bv, gamma):
    g = float(np.asarray(gamma).reshape(-1)[0])
    Wqf = np.asarray(Wq, np.float32)
    Wkf = np.asarray(Wk, np.float32)
    bqf = np.asarray(bq, np.float32)
    bkf = np.asarray(bk, np.float32)
    bvf = np.asarray(bv, np.float32)
    wcat = np.concatenate(
        [
            Wqf.T,
            Wkf.T,
            (g * np.asarray(Wv, np.float32)).T,
            (2.0 * Wqf.T @ bqf)[:, None],
            (2.0 * Wkf.T @ bkf)[:, None],
        ],
        axis=1,
    ).astype(ml_dtypes.bfloat16)
    wcat = np.ascontiguousarray(wcat)
    normc = np.array([bqf @ bqf, bkf @ bkf], np.float32)
    bvg = np.ascontiguousarray(g * bvf, dtype=np.float32)
    bq_col = np.ascontiguousarray(bqf.reshape(CQK, 1), dtype=np.float32)
    mxsel = np.zeros((CQK + 2, CQK + 1), np.float32)
    for i in range(CQK):
        mxsel[i, i] = 1.0
    mxsel[CQK + 1, CQK] = 1.0
    mxrows = np.zeros((2, CQK + 1), np.float32)
    mxrows[0, 0:CQK] = bkf
    mxrows[1, CQK] = 1.0
    nbvrow = np.zeros((1, FD), np.float32)
    nbvrow[0, 0:C] = float(N) * g * bvf

    xf = np.asarray(x, dtype=np.float32).reshape(B, C, N)
    in_maps = []
    for core in range(8):
        b, h = core // 2, core % 2
        xsh = np.ascontiguousarray(
            xf[b, :, h * NSH : (h + 1) * NSH].astype(ml_dtypes.bfloat16)
        )
        in_maps.append(
            {
                "xs": xsh,
                "wcat": wcat,
                "bq": bq_col,
                "normc": normc,
                "bvg": bvg,
                "mxsel": mxsel,
                "mxrows": mxrows,
                "nbvrow": nbvrow,
            }
        )
    return in_maps


def run(inputs, trace=False):
    nc = _get_nc()
    in_maps = _prep_in_maps(**inputs)
    res = bass_utils.run_bass_kernel_spmd(
        nc, in_maps, core_ids=list(range(8)), trace=trace
    )
    outf = np.empty((B, C, N), np.float32)
    for core in range(8):
        b, h = core // 2, core % 2
        outf[b, :, h * NSH : (h + 1) * NSH] = (
            res.results[core]["out"].astype(np.float32).T
        )
    return outf.reshape(B, C, HH, WW), res


def kernel(**inputs):
    out, _ = run(inputs, trace=False)
    return out


# revision 30
# speedup vs baseline: 1.2197x; 1.2197x over previous
"""Trainium2 Bass kernel for linear (taylor/sparse) attention.

Reference computation (per batch b, with xf = x.reshape(b, C, N)):
    Q = Wq@xf + bq            [Cqk, N]
    K = Wk@xf + bk            [Cqk, N]
    V = Wv@xf + bv            [C, N]
    Qh = Q / ||Q||_2 (per position, channel dim)
    Kh = K / ||K||_2
    tailor[n]   = 1 / (N + Qh[:,n] . (sum_n Kh + eps))
    matrix      = Kh @ V^T    [Cqk, C]
    out[:, n]   = gamma * tailor[n] * (sum_n V + matrix^T @ Qh[:,n])

Distribution: 8 cores = 4 batches x 2 halves of N (seq parallel), with a
pairwise AllReduce of the tiny factor.

v3 layout (per core, NSH=8192 positions = 64 tiles of 128):
  A dummy 256B collective fires at t=0 to absorb the CC engine's startup
  and the cross-core launch skew, so the real factor AllReduce later pays
  only the mesh transfer.
  Phase A per tile: fused projection [Qraw|Kraw|V'|2bq.Qraw|2bk.Kraw]
  (322 cols, V' = gamma*Wv@x) into a 6-deep PSUM rotation, Q|K|V copied to
  a persistent SBUF ring (scalar), sums-of-squares accumulated from the
  bf16 ring on DVE (bias-free norms: ||K+bk||^2 = ssk_raw + 2bk.Kraw +
  ||bk||^2, cross terms as extra projection columns), and per 8-tile group
  the K-norm reciprocals scale the kh ring so the factor
      psf[34, 257] = [r*Kraw | r | 1]^T @ [V' | 1]
  accumulates bias-free (bk folds in post-collective as a rank-1 PE
  update: mx = sel^T.facg + bk (x) facg_row32 + N*bv' row).  GpSimd stays
  nearly idle in phase A so the AllReduce dispatches the moment the factor
  is ready; phase B (c-major Q+bq via matmul with per-partition bias, the
  ||Q+bq|| row via sqrt + PE transpose) hides the collective latency.
  Phase 2 per 4 tiles: denominator-column matmuls into a shared PSUM bank
  + one batched reciprocal, then [128, 256] matmuls into a 7-deep PSUM
  rotation, scaled on alternating Scalar/Vector into bf16, DMA'd out 8
  tiles per descriptor.
Output is n-major bf16 [N_shard, C]; the host transposes/casts back.
"""

import ml_dtypes
import numpy as np
from contextlib import ExitStack

import concourse.bass as bass
import concourse.bacc as bacc
import concourse.tile as tile
from concourse import mybir
from concourse import bass_utils
from concourse.masks import make_identity

F32 = mybir.dt.float32
BF16 = mybir.dt.bfloat16
ALU = mybir.AluOpType
ACTF = mybir.ActivationFunctionType

B, C, HH, WW = 4, 256, 128, 128
N = HH * WW            # 16384 positions per batch
NSH = N // 2           # 8192 positions per core
CQK = 32
PW = 2 * CQK + C + 2   # 322: [Q | K | V | qcross | kcross] fused projection
KVW = 2 * CQK + C + 1  # 321: kvring slot = [Q | K | V | ones]
FD = C + 1             # 257: factor free width ([V | 1])
NT512 = NSH // 512     # 16
NT128 = NSH // 128     # 64
GRP = 8                # tiles per K-norm batching group
EPS = 1e-6
RG = [[0, 1], [2, 3], [4, 5], [6, 7]]

_CACHE = {}


def _build():
    nc = bacc.Bacc("TRN2", target_bir_lowering=False, debug=False, num_devices=8)

    xs = nc.dram_tensor("xs", [C, NSH], BF16, kind="ExternalInput").ap()
    wcat = nc.dram_tensor("wcat", [C, PW], BF16, kind="ExternalInput").ap()
    bq_in = nc.dram_tensor("bq", [CQK, 1], F32, kind="ExternalInput").ap()
    normc = nc.dram_tensor("normc", [2], F32, kind="ExternalInput").ap()
    bvg = nc.dram_tensor("bvg", [C], F32, kind="ExternalInput").ap()
    mxsel = nc.dram_tensor("mxsel", [CQK + 2, CQK + 1], F32, kind="ExternalInput").ap()
    mxrows = nc.dram_tensor("mxrows", [2, CQK + 1], F32, kind="ExternalInput").ap()
    nbvrow = nc.dram_tensor("nbvrow", [1, FD], F32, kind="ExternalInput").ap()
    out = nc.dram_tensor("out", [NSH, C], BF16, kind="ExternalOutput").ap()

    with tile.TileContext(nc) as tc, ExitStack() as ctx:
        _body(ctx, tc, nc, xs, wcat, bq_in, normc, bvg, mxsel, mxrows, nbvrow, out)

    nc.compile()
    return nc


def _body(ctx, tc, nc, xs, wcat, bq_in, normc, bvg, mxsel, mxrows, nbvrow, out):
    singles = ctx.enter_context(tc.tile_pool(name="singles", bufs=1))
    xpool = ctx.enter_context(tc.tile_pool(name="x", bufs=NT512))
    smalls = ctx.enter_context(tc.tile_pool(name="smalls", bufs=4))
    scpool = ctx.enter_context(tc.tile_pool(name="scratch", bufs=4))
    outpool = ctx.enter_context(tc.tile_pool(name="outp", bufs=2))
    dram = ctx.enter_context(tc.tile_pool(name="dram", bufs=1, space="DRAM"))

    # ---- warm-up collective: absorbs CC startup + cross-core launch skew
    # so the real factor AllReduce later only pays the mesh transfer ----
    warm_in = dram.tile([1, 64], F32)
    warm_out = dram.tile([2, 64], F32)
    nc.gpsimd.collective_compute(
        "AllGather", ALU.bypass, replica_groups=RG,
        ins=[warm_in.opt()], outs=[warm_out.opt()],
    )
    warm_in2 = dram.tile([1, 64], F32)
    warm_out2 = dram.tile([2, 64], F32)

    # ---- one-time setup (small uploads on gpsimd; it idles afterwards so
    # the factor collective dispatches without queueing) ----
    wcat_sb = singles.tile([128, 2, PW], BF16)
    nc.sync.dma_start(wcat_sb[:], wcat.rearrange("(cb cp) w -> cp cb w", cb=2))
    bq_col = singles.tile([CQK, 1], F32)
    nc.gpsimd.dma_start(bq_col[:], bq_in)
    normc_rep = singles.tile([128, 2], F32)
    nc.gpsimd.dma_start(
        normc_rep[:], normc.unsqueeze(0).partition_broadcast(128).squeeze(1)
    )
    bv_rep = singles.tile([CQK, C], F32)
    nc.gpsimd.dma_start(
        bv_rep[:], bvg.unsqueeze(0).partition_broadcast(CQK).squeeze(1)
    )
    mxsel_sb = singles.tile([CQK + 2, CQK + 1], F32)
    nc.gpsimd.dma_start(mxsel_sb[:], mxsel)
    # rank-1 fold operands must sit at base partition 32 (matmul base
    # partitions are restricted to 0/32/64 and must match between operands)
    bkrow_sb = singles.tile([CQK + 1, CQK + 1], F32)   # row 32: [bk | 0]
    nc.gpsimd.dma_start(bkrow_sb[CQK : CQK + 1, :], mxrows[0:1, :])
    erow_sb = singles.tile([CQK + 1, CQK + 1], F32)    # row 32: [0..0 | 1]
    nc.gpsimd.dma_start(erow_sb[CQK : CQK + 1, :], mxrows[1:2, :])
    nbvrow_sb = singles.tile([CQK + 1, FD], F32)       # row 32: [N*g*bv | 0]
    nc.gpsimd.dma_start(nbvrow_sb[CQK : CQK + 1, :], nbvrow)
    ident = singles.tile([128, 128], F32)
    make_identity(nc, ident[:])

    # persistent rings: ones columns preset once
    kvring = singles.tile([128, 16, KVW], BF16)      # [Q | K | V | ones]
    nc.vector.memset(kvring[:, :, KVW - 1 : KVW], 1.0)
    khring = singles.tile([128, GRP, CQK + 2], BF16)  # [r*K | r | ones]
    nc.vector.memset(khring[:, :, CQK + 1 : CQK + 2], 1.0)

    qx = singles.tile([CQK + 1, NSH], BF16)          # c-major Q+bq rows + ||Q|| row
    ssq_stack = singles.tile([128, NT128], F32)      # sum(Qraw^2), col t
    ssk_stack = singles.tile([128, NT128], F32)      # sum(Kraw^2), col t
    crossqk = singles.tile([128, 2, NT128], F32)     # [2*bq.Qraw ; 2*bk.Kraw]
    cc_in = dram.tile([CQK + 2, FD], F32)
    cc_red = dram.tile([CQK + 2, FD], F32)

    xt_tiles = [None] * NT512

    with tc.tile_pool(name="ps_kqv", bufs=7, space="PSUM") as ps_kqv, tc.tile_pool(
        name="ps_f", bufs=1, space="PSUM"
    ) as ps_f:
        psf = ps_f.tile([CQK + 2, FD], F32)          # factor accumulator
        pending_tail = None

        def emit_tail(g0):
            nsum = smalls.tile([128, GRP], F32)
            nc.gpsimd.tensor_tensor(
                nsum[:], ssk_stack[:, g0 : g0 + GRP], crossqk[:, 1, g0 : g0 + GRP],
                ALU.add,
            )
            normk = smalls.tile([128, GRP], F32)
            nc.scalar.activation(
                normk[:], nsum[:], ACTF.Sqrt, bias=normc_rep[:, 1:2], scale=1.0
            )
            rnorm = smalls.tile([128, GRP], F32)
            nc.vector.reciprocal(rnorm[:], normk[:])
            # r column of the kh ring (slot tt%GRP), bf16 for the factor matmul
            nc.gpsimd.tensor_copy(khring[:, 0:GRP, CQK : CQK + 1], rnorm[:])
            for tt in range(g0, g0 + GRP):
                sl = tt % GRP
                nc.vector.tensor_scalar_mul(
                    khring[:, sl, 0:CQK],
                    kvring[:, tt % 16, CQK : 2 * CQK],
                    rnorm[:, sl : sl + 1],
                )
                nc.tensor.matmul(
                    psf[:], khring[:, sl, :], kvring[:, tt % 16, 2 * CQK : KVW],
                    start=(tt == 0), stop=(tt == NT128 - 1),
                )

        # ---- phase A: projections + norm accums + factor ----
        for j in range(NT512):
            if j == NT512 // 2:
                # re-align the cores mid-phase so the factor collective
                # below sees minimal peer skew
                nc.gpsimd.collective_compute(
                    "AllGather", ALU.bypass, replica_groups=RG,
                    ins=[warm_in2.opt()], outs=[warm_out2.opt()],
                )
            xt = xpool.tile([128, 2, 512], BF16)
            nc.sync.dma_start(
                xt[:],
                xs.rearrange("(cb cp) n -> cp cb n", cb=2)[
                    :, :, j * 512 : (j + 1) * 512
                ],
            )
            xt_tiles[j] = xt

            for u in range(4):
                t = j * 4 + u
                if u == 2 and j % 2 == 0 and pending_tail is not None:
                    emit_tail(pending_tail)
                    pending_tail = None
                pskqv = ps_kqv.tile([128, PW], F32)
                for cb in range(2):
                    nc.tensor.matmul(
                        pskqv[:], xt[:, cb, u * 128 : (u + 1) * 128],
                        wcat_sb[:, cb, :],
                        start=(cb == 0), stop=(cb == 1),
                    )
                # Q|K -> ring on vector, V -> ring on scalar (bf16 casts);
                # squares accumulate from the SBUF bf16 copy on gpsimd
                # (engines read only one PSUM input; gpsimd reads none)
                nc.vector.tensor_copy(
                    kvring[:, t % 16, 0 : 2 * CQK], pskqv[:, 0 : 2 * CQK]
                )
                nc.scalar.copy(
                    kvring[:, t % 16, 2 * CQK : 2 * CQK + C],
                    pskqv[:, 2 * CQK : 2 * CQK + C],
                )
                scr_q = scpool.tile([128, CQK], BF16)
                scr_k = scpool.tile([128, CQK], BF16)
                nc.vector.scalar_tensor_tensor(
                    scr_q[:], kvring[:, t % 16, 0:CQK], 1.0,
                    kvring[:, t % 16, 0:CQK],
                    ALU.mult, ALU.mult, accum_out=ssq_stack[:, t : t + 1],
                )
                nc.vector.scalar_tensor_tensor(
                    scr_k[:],
                    kvring[:, t % 16, CQK : 2 * CQK],
                    1.0,
                    kvring[:, t % 16, CQK : 2 * CQK],
                    ALU.mult, ALU.mult, accum_out=ssk_stack[:, t : t + 1],
                )
                nc.vector.tensor_copy(
                    crossqk[:, :, t : t + 1], pskqv[:, PW - 2 : PW]
                )

            if (j + 1) % (GRP // 4) == 0:
                pending_tail = (j + 1) * 4 - GRP
        if pending_tail is not None:
            emit_tail(pending_tail)
            pending_tail = None

        # ---- collective: fire as soon as the factor is done ----
        fac_loc = singles.tile([CQK + 2, FD], F32)
        nc.vector.tensor_copy(fac_loc[:], psf[:])
        nc.sync.dma_start(cc_in[:], fac_loc[:])
        nc.gpsimd.collective_compute(
            "AllReduce", ALU.add, replica_groups=RG,
            ins=[cc_in.opt()], outs=[cc_red.opt()],
        )

    with tc.tile_pool(name="ps_b", bufs=4, space="PSUM") as ps_b:
        # ---- phase B (hides the collective): c-major Q+bq, ||Q+bq|| row ----
        for j in range(NT512):
            psq = ps_b.tile([CQK, 512], F32, tag="shared")
            for cb in range(2):
                nc.tensor.matmul(
                    psq[:], wcat_sb[:, cb, 0:CQK], xt_tiles[j][:, cb, :],
                    start=(cb == 0), stop=(cb == 1),
                )
            if j % 8 < 5:
                nc.scalar.activation(
                    qx[0:CQK, j * 512 : (j + 1) * 512], psq[:],
                    ACTF.Identity, bias=bq_col[:], scale=1.0,
                )
            else:
                nc.vector.tensor_scalar_add(
                    qx[0:CQK, j * 512 : (j + 1) * 512], psq[:], bq_col[:]
                )

        ssq_tot = singles.tile([128, NT128], F32)
        nc.vector.tensor_tensor(ssq_tot[:], ssq_stack[:], crossqk[:, 0, :], ALU.add)
        normq_stack = singles.tile([128, NT128], F32)
        nc.scalar.activation(
            normq_stack[:], ssq_tot[:], ACTF.Sqrt, bias=normc_rep[:, 0:1], scale=1.0
        )
        pst = ps_b.tile([NT128, 128], F32, tag="shared")
        nc.tensor.transpose(pst[:], normq_stack[:], ident[:])
        trT = singles.tile([NT128, 128], BF16)
        nc.vector.tensor_copy(trT[:], pst[:])
        row_scratch = dram.tile([NT128, 128], BF16)
        nc.sync.dma_start(row_scratch[:], trT[:])
        nc.sync.dma_start(
            qx[CQK : CQK + 1, :],
            row_scratch[:].rearrange("a b -> (a b)").unsqueeze(0),
        )

        # ---- post-collective: facg + mx build (PE rank-1 folds) ----
        facg = singles.tile([CQK + 2, FD], F32)
        nc.sync.dma_start(facg[:], cc_red[:])
        ps_mx = ps_b.tile([CQK + 1, FD], F32, tag="shared")
        nc.tensor.matmul(ps_mx[:], mxsel_sb[:], facg[:], start=True, stop=False)
        nc.tensor.matmul(
            ps_mx[:],
            bkrow_sb[CQK : CQK + 1, :],
            facg[CQK : CQK + 1, :],
            start=False,
            stop=False,
        )
        nc.tensor.matmul(
            ps_mx[:],
            erow_sb[CQK : CQK + 1, :],
            nbvrow_sb[CQK : CQK + 1, :],
            start=False,
            stop=True,
        )
        mx = singles.tile([CQK + 1, FD], BF16)
        # rows 0:32: matrix + Ksum (x) bv'  (Ksum = ps_mx col 256)
        nc.vector.scalar_tensor_tensor(
            mx[0:CQK, 0:C], bv_rep[:], ps_mx[0:CQK, C : C + 1], ps_mx[0:CQK, 0:C],
            ALU.mult, ALU.add,
        )
        nc.vector.tensor_scalar_add(
            mx[0:CQK, C : C + 1], ps_mx[0:CQK, C : C + 1], EPS
        )
        nc.vector.tensor_copy(mx[CQK : CQK + 1, :], ps_mx[CQK : CQK + 1, :])

        # ---- phase 2 ----
        with tc.tile_pool(name="ps_p2", bufs=3, space="PSUM") as ps_p2:
            out8 = out.rearrange("(g u p) c -> g p u c", u=GRP, p=128)
            for g8 in range(NT128 // GRP):
                ot = outpool.tile([128, GRP, C], BF16)
                for half in range(2):
                    g4 = g8 * 2 + half
                    psden = ps_p2.tile([128, 4], F32, tag="den", bufs=1)
                    for i in range(4):
                        t = g4 * 4 + i
                        nc.tensor.matmul(
                            psden[:, i : i + 1],
                            qx[:, t * 128 : (t + 1) * 128],
                            mx[:, C : C + 1],
                            start=True, stop=True,
                        )
                    rec4 = smalls.tile([128, 4], F32)
                    nc.vector.reciprocal(rec4[:], psden[:])
                    for i in range(4):
                        t = g4 * 4 + i
                        u = half * 4 + i
                        if i % 2 == 0:
                            ps2 = ps_b.tile([128, C], F32, tag="shared")
                        else:
                            ps2 = ps_p2.tile([128, C], F32, tag="p2")
                        nc.tensor.matmul(
                            ps2[:], qx[:, t * 128 : (t + 1) * 128], mx[:, 0:C],
                            start=True, stop=True,
                        )
                        # scale split by columns across both engines
                        nc.vector.tensor_scalar_mul(
                            ot[:, u, 0 : C // 2], ps2[:, 0 : C // 2],
                            rec4[:, i : i + 1],
                        )
                        nc.scalar.mul(
                            ot[:, u, C // 2 : C], ps2[:, C // 2 : C],
                            rec4[:, i : i + 1],
                        )
                nc.sync.dma_start(out8[g8], ot[:])


def _get_nc():
    if "nc" not in _CACHE:
        _CACHE["nc"] = _build()
    return _CACHE["nc"]


def _prep_in_maps(x, Wq, bq, Wk, bk, Wv, bv, gamma):
    g = float(np.asarray(gamma).reshape(-1)[0])
    Wqf = np.asarray(Wq, np.float32)
    Wkf = np.asarray(Wk, np.float32)
    bqf = np.asarray(bq, np.float32)
    bkf = np.asarray(bk, np.float32)
    bvf = np.asarray(bv, np.float32)
    wcat = np.concatenate(
        [
            Wqf.T,
            Wkf.T,
            (g * np.asarray(Wv, np.float32)).T,
            (2.0 * Wqf.T @ bqf)[:, None],
            (2.0 * Wkf.T @ bkf)[:, None],
        ],
        axis=1,
    ).astype(ml_dtypes.bfloat16)
    wcat = np.ascontiguousarray(wcat)
    normc = np.array([bqf @ bqf, bkf @ bkf], np.float32)
    bvg = np.ascontiguousarray(g * bvf, dtype=np.float32)
    bq_col = np.ascontiguousarray(bqf.reshape(CQK, 1), dtype=np.float32)
    mxsel = np.zeros((CQK + 2, CQK + 1), np.float32)
    for i in range(CQK):
        mxsel[i, i] = 1.0
    mxsel[CQK + 1, CQK] = 1.0
    mxrows = np.zeros((2, CQK + 1), np.float32)
    mxrows[0, 0:CQK] = bkf
    mxrows[1, CQK] = 1.0
    nbvrow = np.zeros((1, FD), np.float32)
    nbvrow[0, 0:C] = float(N) * g * bvf

    xf = np.asarray(x, dtype=np.float32).reshape(B, C, N)
    in_maps = []
    for core in range(8):
        b, h = core // 2, core % 2
        xsh = np.ascontiguousarray(
            xf[b, :, h * NSH : (h + 1) * NSH].astype(ml_dtypes.bfloat16)
        )
        in_maps.append(
            {
                "xs": xsh,
                "wcat": wcat,
                "bq": bq_col,
                "normc": normc,
                "bvg": bvg,
                "mxsel": mxsel,
                "mxrows": mxrows,
                "nbvrow": nbvrow,
            }
        )
    return in_maps


def run(inputs, trace=False):
    nc = _get_nc()
    in_maps = _prep_in_maps(**inputs)
    res = bass_utils.run_bass_kernel_spmd(
        nc, in_maps, core_ids=list(range(8)), trace=trace
    )
    outf = np.empty((B, C, N), np.float32)
    for core in range(8):
        b, h = core // 2, core % 2
        outf[b, :, h * NSH : (h + 1) * NSH] = (
            res.results[core]["out"].astype(np.float32).T
        )
    return outf.reshape(B, C, HH, WW), res


def kernel(**inputs):
    out, _ = run(inputs, trace=False)
    return out


# revision 31
# speedup vs baseline: 1.2385x; 1.0154x over previous
"""Trainium2 Bass kernel for linear (taylor/sparse) attention.

Reference computation (per batch b, with xf = x.reshape(b, C, N)):
    Q = Wq@xf + bq            [Cqk, N]
    K = Wk@xf + bk            [Cqk, N]
    V = Wv@xf + bv            [C, N]
    Qh = Q / ||Q||_2 (per position, channel dim)
    Kh = K / ||K||_2
    tailor[n]   = 1 / (N + Qh[:,n] . (sum_n Kh + eps))
    matrix      = Kh @ V^T    [Cqk, C]
    out[:, n]   = gamma * tailor[n] * (sum_n V + matrix^T @ Qh[:,n])

Distribution: 8 cores = 4 batches x 2 halves of N (seq parallel), with a
pairwise AllReduce of the tiny factor.

v3 layout (per core, NSH=8192 positions = 64 tiles of 128):
  A dummy 256B collective fires at t=0 to absorb the CC engine's startup
  and the cross-core launch skew, so the real factor AllReduce later pays
  only the mesh transfer.
  Phase A per tile: fused projection [Qraw|Kraw|V'|2bq.Qraw|2bk.Kraw]
  (322 cols, V' = gamma*Wv@x) into a 6-deep PSUM rotation, Q|K|V copied to
  a persistent SBUF ring (scalar), sums-of-squares accumulated from the
  bf16 ring on DVE (bias-free norms: ||K+bk||^2 = ssk_raw + 2bk.Kraw +
  ||bk||^2, cross terms as extra projection columns), and per 8-tile group
  the K-norm reciprocals scale the kh ring so the factor
      psf[34, 257] = [r*Kraw | r | 1]^T @ [V' | 1]
  accumulates bias-free (bk folds in post-collective as a rank-1 PE
  update: mx = sel^T.facg + bk (x) facg_row32 + N*bv' row).  GpSimd stays
  nearly idle in phase A so the AllReduce dispatches the moment the factor
  is ready; phase B (c-major Q+bq via matmul with per-partition bias, the
  ||Q+bq|| row via sqrt + PE transpose) hides the collective latency.
  Phase 2 per 4 tiles: denominator-column matmuls into a shared PSUM bank
  + one batched reciprocal, then [128, 256] matmuls into a 7-deep PSUM
  rotation, scaled on alternating Scalar/Vector into bf16, DMA'd out 8
  tiles per descriptor.
Output is n-major bf16 [N_shard, C]; the host transposes/casts back.
"""

import ml_dtypes
import numpy as np
from contextlib import ExitStack

import concourse.bass as bass
import concourse.bacc as bacc
import concourse.tile as tile
from concourse import mybir
from concourse import bass_utils
from concourse.masks import make_identity

F32 = mybir.dt.float32
BF16 = mybir.dt.bfloat16
ALU = mybir.AluOpType
ACTF = mybir.ActivationFunctionType

B, C, HH, WW = 4, 256, 128, 128
N = HH * WW            # 16384 positions per batch
NSH = N // 2           # 8192 positions per core
CQK = 32
PW = 2 * CQK + C + 2   # 322: [Q | K | V | qcross | kcross] fused projection
KVW = 2 * CQK + C + 1  # 321: kvring slot = [Q | K | V | ones]
FD = C + 1             # 257: factor free width ([V | 1])
NT512 = NSH // 512     # 16
NT128 = NSH // 128     # 64
GRP = 8                # tiles per K-norm batching group
EPS = 1e-6
RG = [[0, 1], [2, 3], [4, 5], [6, 7]]

_CACHE = {}


def _build():
    nc = bacc.Bacc("TRN2", target_bir_lowering=False, debug=False, num_devices=8)

    xs = nc.dram_tensor("xs", [C, NSH], BF16, kind="ExternalInput").ap()
    wcat = nc.dram_tensor("wcat", [C, PW], BF16, kind="ExternalInput").ap()
    bq_in = nc.dram_tensor("bq", [CQK, 1], F32, kind="ExternalInput").ap()
    normc = nc.dram_tensor("normc", [2], F32, kind="ExternalInput").ap()
    bvg = nc.dram_tensor("bvg", [C], F32, kind="ExternalInput").ap()
    mxsel = nc.dram_tensor("mxsel", [CQK + 2, CQK + 1], F32, kind="ExternalInput").ap()
    mxrows = nc.dram_tensor("mxrows", [2, CQK + 1], F32, kind="ExternalInput").ap()
    nbvrow = nc.dram_tensor("nbvrow", [1, FD], F32, kind="ExternalInput").ap()
    out = nc.dram_tensor("out", [NSH, C], BF16, kind="ExternalOutput").ap()

    with tile.TileContext(nc) as tc, ExitStack() as ctx:
        _body(ctx, tc, nc, xs, wcat, bq_in, normc, bvg, mxsel, mxrows, nbvrow, out)

    nc.compile()
    return nc


def _body(ctx, tc, nc, xs, wcat, bq_in, normc, bvg, mxsel, mxrows, nbvrow, out):
    singles = ctx.enter_context(tc.tile_pool(name="singles", bufs=1))
    xpool = ctx.enter_context(tc.tile_pool(name="x", bufs=NT512))
    smalls = ctx.enter_context(tc.tile_pool(name="smalls", bufs=4))
    scpool = ctx.enter_context(tc.tile_pool(name="scratch", bufs=4))
    outpool = ctx.enter_context(tc.tile_pool(name="outp", bufs=2))
    dram = ctx.enter_context(tc.tile_pool(name="dram", bufs=1, space="DRAM"))

    # ---- warm-up collective: absorbs CC startup + cross-core launch skew
    # so the real factor AllReduce later only pays the mesh transfer ----
    warm_in = dram.tile([1, 64], F32)
    warm_out = dram.tile([2, 64], F32)
    nc.gpsimd.collective_compute(
        "AllGather", ALU.bypass, replica_groups=RG,
        ins=[warm_in.opt()], outs=[warm_out.opt()],
    )
    warm_in2 = dram.tile([1, 64], F32)
    warm_out2 = dram.tile([2, 64], F32)

    # ---- one-time setup (small uploads on gpsimd; it idles afterwards so
    # the factor collective dispatches without queueing) ----
    wcat_sb = singles.tile([128, 2, PW], BF16)
    nc.sync.dma_start(wcat_sb[:], wcat.rearrange("(cb cp) w -> cp cb w", cb=2))
    bq_col = singles.tile([CQK, 1], F32)
    nc.gpsimd.dma_start(bq_col[:], bq_in)
    normc_rep = singles.tile([128, 2], F32)
    nc.gpsimd.dma_start(
        normc_rep[:], normc.unsqueeze(0).partition_broadcast(128).squeeze(1)
    )
    bv_rep = singles.tile([CQK, C], F32)
    nc.gpsimd.dma_start(
        bv_rep[:], bvg.unsqueeze(0).partition_broadcast(CQK).squeeze(1)
    )
    mxsel_sb = singles.tile([CQK + 2, CQK + 1], F32)
    nc.gpsimd.dma_start(mxsel_sb[:], mxsel)
    # rank-1 fold operands must sit at base partition 32 (matmul base
    # partitions are restricted to 0/32/64 and must match between operands)
    bkrow_sb = singles.tile([CQK + 1, CQK + 1], F32)   # row 32: [bk | 0]
    nc.gpsimd.dma_start(bkrow_sb[CQK : CQK + 1, :], mxrows[0:1, :])
    erow_sb = singles.tile([CQK + 1, CQK + 1], F32)    # row 32: [0..0 | 1]
    nc.gpsimd.dma_start(erow_sb[CQK : CQK + 1, :], mxrows[1:2, :])
    nbvrow_sb = singles.tile([CQK + 1, FD], F32)       # row 32: [N*g*bv | 0]
    nc.gpsimd.dma_start(nbvrow_sb[CQK : CQK + 1, :], nbvrow)
    ident = singles.tile([128, 128], F32)
    make_identity(nc, ident[:])

    # persistent rings: ones columns preset once
    kvring = singles.tile([128, 16, KVW], BF16)      # [Q | K | V | ones]
    nc.vector.memset(kvring[:, :, KVW - 1 : KVW], 1.0)
    khring = singles.tile([128, GRP, CQK + 2], BF16)  # [r*K | r | ones]
    nc.vector.memset(khring[:, :, CQK + 1 : CQK + 2], 1.0)

    qx = singles.tile([CQK + 1, NSH], BF16)          # c-major Q+bq rows + ||Q|| row
    ssq_stack = singles.tile([128, NT128], F32)      # sum(Qraw^2), col t
    ssk_stack = singles.tile([128, NT128], F32)      # sum(Kraw^2), col t
    crossqk = singles.tile([128, 2, NT128], F32)     # [2*bq.Qraw ; 2*bk.Kraw]
    cc_in = dram.tile([CQK + 2, FD], F32)
    cc_red = dram.tile([CQK + 2, FD], F32)

    xt_tiles = [None] * NT512

    with tc.tile_pool(name="ps_kqv", bufs=7, space="PSUM") as ps_kqv, tc.tile_pool(
        name="ps_f", bufs=1, space="PSUM"
    ) as ps_f:
        psf = ps_f.tile([CQK + 2, FD], F32)          # factor accumulator
        pending_tail = None

        def emit_tail(g0):
            nsum = smalls.tile([128, GRP], F32)
            nc.gpsimd.tensor_tensor(
                nsum[:], ssk_stack[:, g0 : g0 + GRP], crossqk[:, 1, g0 : g0 + GRP],
                ALU.add,
            )
            normk = smalls.tile([128, GRP], F32)
            nc.scalar.activation(
                normk[:], nsum[:], ACTF.Sqrt, bias=normc_rep[:, 1:2], scale=1.0
            )
            rnorm = smalls.tile([128, GRP], F32)
            nc.vector.reciprocal(rnorm[:], normk[:])
            # r column of the kh ring (slot tt%GRP), bf16 for the factor matmul
            nc.gpsimd.tensor_copy(khring[:, 0:GRP, CQK : CQK + 1], rnorm[:])
            for tt in range(g0, g0 + GRP):
                sl = tt % GRP
                nc.vector.tensor_scalar_mul(
                    khring[:, sl, 0:CQK],
                    kvring[:, tt % 16, CQK : 2 * CQK],
                    rnorm[:, sl : sl + 1],
                )
                nc.tensor.matmul(
                    psf[:], khring[:, sl, :], kvring[:, tt % 16, 2 * CQK : KVW],
                    start=(tt == 0), stop=(tt == NT128 - 1),
                )

        # ---- phase A: projections + norm accums + factor ----
        for j in range(NT512):
            xt = xpool.tile([128, 2, 512], BF16)
            nc.sync.dma_start(
                xt[:],
                xs.rearrange("(cb cp) n -> cp cb n", cb=2)[
                    :, :, j * 512 : (j + 1) * 512
                ],
            )
            xt_tiles[j] = xt

            for u in range(4):
                t = j * 4 + u
                if u == 2 and j % 2 == 0 and pending_tail is not None:
                    emit_tail(pending_tail)
                    pending_tail = None
                pskqv = ps_kqv.tile([128, PW], F32)
                for cb in range(2):
                    nc.tensor.matmul(
                        pskqv[:], xt[:, cb, u * 128 : (u + 1) * 128],
                        wcat_sb[:, cb, :],
                        start=(cb == 0), stop=(cb == 1),
                    )
                # Q|K -> ring on vector, V -> ring on scalar (bf16 casts);
                # squares accumulate from the SBUF bf16 copy on gpsimd
                # (engines read only one PSUM input; gpsimd reads none)
                nc.vector.tensor_copy(
                    kvring[:, t % 16, 0 : 2 * CQK], pskqv[:, 0 : 2 * CQK]
                )
                nc.scalar.copy(
                    kvring[:, t % 16, 2 * CQK : 2 * CQK + C],
                    pskqv[:, 2 * CQK : 2 * CQK + C],
                )
                scr_q = scpool.tile([128, CQK], BF16)
                scr_k = scpool.tile([128, CQK], BF16)
                nc.vector.scalar_tensor_tensor(
                    scr_q[:], kvring[:, t % 16, 0:CQK], 1.0,
                    kvring[:, t % 16, 0:CQK],
                    ALU.mult, ALU.mult, accum_out=ssq_stack[:, t : t + 1],
                )
                nc.vector.scalar_tensor_tensor(
                    scr_k[:],
                    kvring[:, t % 16, CQK : 2 * CQK],
                    1.0,
                    kvring[:, t % 16, CQK : 2 * CQK],
                    ALU.mult, ALU.mult, accum_out=ssk_stack[:, t : t + 1],
                )
                nc.vector.tensor_copy(
                    crossqk[:, :, t : t + 1], pskqv[:, PW - 2 : PW]
                )

            if (j + 1) % (GRP // 4) == 0:
                pending_tail = (j + 1) * 4 - GRP
        if pending_tail is not None:
            emit_tail(pending_tail)
            pending_tail = None

        # ---- collective: fire as soon as the factor is done ----
        fac_loc = singles.tile([CQK + 2, FD], F32)
        nc.vector.tensor_copy(fac_loc[:], psf[:])
        nc.sync.dma_start(cc_in[:], fac_loc[:])
        nc.gpsimd.collective_compute(
            "AllReduce", ALU.add, replica_groups=RG,
            ins=[cc_in.opt()], outs=[cc_red.opt()],
        )

    with tc.tile_pool(name="ps_b", bufs=4, space="PSUM") as ps_b:
        # ---- phase B (hides the collective): c-major Q+bq, ||Q+bq|| row ----
        for j in range(NT512):
            psq = ps_b.tile([CQK, 512], F32, tag="shared")
            for cb in range(2):
                nc.tensor.matmul(
                    psq[:], wcat_sb[:, cb, 0:CQK], xt_tiles[j][:, cb, :],
                    start=(cb == 0), stop=(cb == 1),
                )
            if j % 8 < 5:
                nc.scalar.activation(
                    qx[0:CQK, j * 512 : (j + 1) * 512], psq[:],
                    ACTF.Identity, bias=bq_col[:], scale=1.0,
                )
            else:
                nc.vector.tensor_scalar_add(
                    qx[0:CQK, j * 512 : (j + 1) * 512], psq[:], bq_col[:]
                )

        ssq_tot = singles.tile([128, NT128], F32)
        nc.vector.tensor_tensor(ssq_tot[:], ssq_stack[:], crossqk[:, 0, :], ALU.add)
        normq_stack = singles.tile([128, NT128], F32)
        nc.scalar.activation(
            normq_stack[:], ssq_tot[:], ACTF.Sqrt, bias=normc_rep[:, 0:1], scale=1.0
        )
        pst = ps_b.tile([NT128, 128], F32, tag="shared")
        nc.tensor.transpose(pst[:], normq_stack[:], ident[:])
        trT = singles.tile([NT128, 128], BF16)
        nc.vector.tensor_copy(trT[:], pst[:])
        row_scratch = dram.tile([NT128, 128], BF16)
        nc.sync.dma_start(row_scratch[:], trT[:])
        nc.sync.dma_start(
            qx[CQK : CQK + 1, :],
            row_scratch[:].rearrange("a b -> (a b)").unsqueeze(0),
        )

        # ---- post-collective: facg + mx build (PE rank-1 folds) ----
        facg = singles.tile([CQK + 2, FD], F32)
        nc.sync.dma_start(facg[:], cc_red[:])
        ps_mx = ps_b.tile([CQK + 1, FD], F32, tag="shared")
        nc.tensor.matmul(ps_mx[:], mxsel_sb[:], facg[:], start=True, stop=False)
        nc.tensor.matmul(
            ps_mx[:],
            bkrow_sb[CQK : CQK + 1, :],
            facg[CQK : CQK + 1, :],
            start=False,
            stop=False,
        )
        nc.tensor.matmul(
            ps_mx[:],
            erow_sb[CQK : CQK + 1, :],
            nbvrow_sb[CQK : CQK + 1, :],
            start=False,
            stop=True,
        )
        mx = singles.tile([CQK + 1, FD], BF16)
        # rows 0:32: matrix + Ksum (x) bv'  (Ksum = ps_mx col 256)
        nc.vector.scalar_tensor_tensor(
            mx[0:CQK, 0:C], bv_rep[:], ps_mx[0:CQK, C : C + 1], ps_mx[0:CQK, 0:C],
            ALU.mult, ALU.add,
        )
        nc.vector.tensor_scalar_add(
            mx[0:CQK, C : C + 1], ps_mx[0:CQK, C : C + 1], EPS
        )
        nc.vector.tensor_copy(mx[CQK : CQK + 1, :], ps_mx[CQK : CQK + 1, :])

        # ---- phase 2 ----
        with tc.tile_pool(name="ps_p2", bufs=3, space="PSUM") as ps_p2:
            out8 = out.rearrange("(g u p) c -> g p u c", u=GRP, p=128)
            for g8 in range(NT128 // GRP):
                ot = outpool.tile([128, GRP, C], BF16)
                for half in range(2):
                    g4 = g8 * 2 + half
                    psden = ps_p2.tile([128, 4], F32, tag="den", bufs=1)
                    for i in range(4):
                        t = g4 * 4 + i
                        nc.tensor.matmul(
                            psden[:, i : i + 1],
                            qx[:, t * 128 : (t + 1) * 128],
                            mx[:, C : C + 1],
                            start=True, stop=True,
                        )
                    rec4 = smalls.tile([128, 4], F32)
                    nc.vector.reciprocal(rec4[:], psden[:])
                    for i in range(4):
                        t = g4 * 4 + i
                        u = half * 4 + i
                        if i % 2 == 0:
                            ps2 = ps_b.tile([128, C], F32, tag="shared")
                        else:
                            ps2 = ps_p2.tile([128, C], F32, tag="p2")
                        nc.tensor.matmul(
                            ps2[:], qx[:, t * 128 : (t + 1) * 128], mx[:, 0:C],
                            start=True, stop=True,
                        )
                        # scale split by columns across both engines
                        nc.vector.tensor_scalar_mul(
                            ot[:, u, 0 : C // 2], ps2[:, 0 : C // 2],
                            rec4[:, i : i + 1],
                        )
                        nc.scalar.mul(
                            ot[:, u, C // 2 : C], ps2[:, C // 2 : C],
                            rec4[:, i : i + 1],
                        )
                nc.sync.dma_start(out8[g8], ot[:])


def _get_nc():
    if "nc" not in _CACHE:
        _CACHE["nc"] = _build()
    return _CACHE["nc"]


def _prep_in_maps(x, Wq, bq, Wk, bk, Wv, bv, gamma):
    g = float(np.asarray(gamma).reshape(-1)[0])
    Wqf = np.asarray(Wq, np.float32)
    Wkf = np.asarray(Wk, np.float32)
    bqf = np.asarray(bq, np.float32)
    bkf = np.asarray(bk, np.float32)
    bvf = np.asarray(bv, np.float32)
    wcat = np.concatenate(
        [
            Wqf.T,
            Wkf.T,
            (g * np.asarray(Wv, np.float32)).T,
            (2.0 * Wqf.T @ bqf)[:, None],
            (2.0 * Wkf.T @ bkf)[:, None],
        ],
        axis=1,
    ).astype(ml_dtypes.bfloat16)
    wcat = np.ascontiguousarray(wcat)
    normc = np.array([bqf @ bqf, bkf @ bkf], np.float32)
    bvg = np.ascontiguousarray(g * bvf, dtype=np.float32)
    bq_col = np.ascontiguousarray(bqf.reshape(CQK, 1), dtype=np.float32)
    mxsel = np.zeros((CQK + 2, CQK + 1), np.float32)
    for i in range(CQK):
        mxsel[i, i] = 1.0
    mxsel[CQK + 1, CQK] = 1.0
    mxrows = np.zeros((2, CQK + 1), np.float32)
    mxrows[0, 0:CQK] = bkf
    mxrows[1, CQK] = 1.0
    nbvrow = np.zeros((1, FD), np.float32)
    nbvrow[0, 0:C] = float(N) * g * bvf

    xf = np.asarray(x, dtype=np.float32).reshape(B, C, N)
    in_maps = []
    for core in range(8):
        b, h = core // 2, core % 2
        xsh = np.ascontiguousarray(
            xf[b, :, h * NSH : (h + 1) * NSH].astype(ml_dtypes.bfloat16)
        )
        in_maps.append(
            {
                "xs": xsh,
                "wcat": wcat,
                "bq": bq_col,
                "normc": normc,
                "bvg": bvg,
                "mxsel": mxsel,
                "mxrows": mxrows,
                "nbvrow": nbvrow,
            }
        )
    return in_maps


def run(inputs, trace=False):
    nc = _get_nc()
    in_maps = _prep_in_maps(**inputs)
    res = bass_utils.run_bass_kernel_spmd(
        nc, in_maps, core_ids=list(range(8)), trace=trace
    )
    outf = np.empty((B, C, N), np.float32)
    for core in range(8):
        b, h = core // 2, core % 2
        outf[b, :, h * NSH : (h + 1) * NSH] = (
            res.results[core]["out"].astype(np.float32).T
        )
    return outf.reshape(B, C, HH, WW), res


def kernel(**inputs):
    out, _ = run(inputs, trace=False)
    return out


# revision 34
# speedup vs baseline: 1.3625x; 1.1002x over previous
"""Trainium2 Bass kernel for linear (taylor/sparse) attention.

Reference computation (per batch b, with xf = x.reshape(b, C, N)):
    Q = Wq@xf + bq            [Cqk, N]
    K = Wk@xf + bk            [Cqk, N]
    V = Wv@xf + bv            [C, N]
    Qh = Q / ||Q||_2 (per position, channel dim)
    Kh = K / ||K||_2
    tailor[n]   = 1 / (N + Qh[:,n] . (sum_n Kh + eps))
    matrix      = Kh @ V^T    [Cqk, C]
    out[:, n]   = gamma * tailor[n] * (sum_n V + matrix^T @ Qh[:,n])

Distribution: 8 cores = 4 batches x 2 halves of N (seq parallel), with a
pairwise AllReduce of the tiny factor.

v3 layout (per core, NSH=8192 positions = 64 tiles of 128):
  A dummy 256B collective fires at t=0 to absorb the CC engine's startup
  and the cross-core launch skew, so the real factor AllReduce later pays
  only the mesh transfer.
  Phase A per tile: fused projection [Qraw|Kraw|V'|2bq.Qraw|2bk.Kraw]
  (322 cols, V' = gamma*Wv@x) into a 6-deep PSUM rotation, Q|K|V copied to
  a persistent SBUF ring (scalar), sums-of-squares accumulated from the
  bf16 ring on DVE (bias-free norms: ||K+bk||^2 = ssk_raw + 2bk.Kraw +
  ||bk||^2, cross terms as extra projection columns), and per 8-tile group
  the K-norm reciprocals scale the kh ring so the factor
      psf[34, 257] = [r*Kraw | r | 1]^T @ [V' | 1]
  accumulates bias-free (bk folds in post-collective as a rank-1 PE
  update: mx = sel^T.facg + bk (x) facg_row32 + N*bv' row).  GpSimd stays
  nearly idle in phase A so the AllReduce dispatches the moment the factor
  is ready; phase B (c-major Q+bq via matmul with per-partition bias, the
  ||Q+bq|| row via sqrt + PE transpose) hides the collective latency.
  Phase 2 per 4 tiles: denominator-column matmuls into a shared PSUM bank
  + one batched reciprocal, then [128, 256] matmuls into a 7-deep PSUM
  rotation, scaled on alternating Scalar/Vector into bf16, DMA'd out 8
  tiles per descriptor.
Output is n-major bf16 [N_shard, C]; the host transposes/casts back.
"""

import ml_dtypes
import numpy as np
from contextlib import ExitStack

import concourse.bass as bass
import concourse.bacc as bacc
import concourse.tile as tile
from concourse import mybir
from concourse import bass_utils
from concourse.masks import make_identity

F32 = mybir.dt.float32
BF16 = mybir.dt.bfloat16
ALU = mybir.AluOpType
ACTF = mybir.ActivationFunctionType

B, C, HH, WW = 4, 256, 128, 128
N = HH * WW            # 16384 positions per batch
NSH = N // 2           # 8192 positions per core
CQK = 32
PW = 2 * CQK + C + 2   # 322: [Q | K | V | qcross | kcross] fused projection
KVW = 2 * CQK + C + 1  # 321: kvring slot = [Q | K | V | ones]
FD = C + 1             # 257: factor free width ([V | 1])
NT512 = NSH // 512     # 16
NT128 = NSH // 128     # 64
GRP = 8                # tiles per K-norm batching group
EPS = 1e-6
RG = [[0, 1], [2, 3], [4, 5], [6, 7]]

_CACHE = {}


def _build():
    nc = bacc.Bacc("TRN2", target_bir_lowering=False, debug=False, num_devices=8)

    xs = nc.dram_tensor("xs", [C, NSH], BF16, kind="ExternalInput").ap()
    wcat = nc.dram_tensor("wcat", [C, PW], BF16, kind="ExternalInput").ap()
    bq_in = nc.dram_tensor("bq", [CQK, 1], F32, kind="ExternalInput").ap()
    normc = nc.dram_tensor("normc", [2], F32, kind="ExternalInput").ap()
    bvg = nc.dram_tensor("bvg", [C], F32, kind="ExternalInput").ap()
    mxsel = nc.dram_tensor("mxsel", [CQK + 2, CQK + 1], F32, kind="ExternalInput").ap()
    mxrows = nc.dram_tensor("mxrows", [2, CQK + 1], F32, kind="ExternalInput").ap()
    nbvrow = nc.dram_tensor("nbvrow", [1, FD], F32, kind="ExternalInput").ap()
    out = nc.dram_tensor("out", [NSH, C], BF16, kind="ExternalOutput").ap()

    with tile.TileContext(nc) as tc, ExitStack() as ctx:
        _body(ctx, tc, nc, xs, wcat, bq_in, normc, bvg, mxsel, mxrows, nbvrow, out)

    nc.compile()
    return nc


def _body(ctx, tc, nc, xs, wcat, bq_in, normc, bvg, mxsel, mxrows, nbvrow, out):
    singles = ctx.enter_context(tc.tile_pool(name="singles", bufs=1))
    xpool = ctx.enter_context(tc.tile_pool(name="x", bufs=NT512))
    smalls = ctx.enter_context(tc.tile_pool(name="smalls", bufs=4))
    scpool = ctx.enter_context(tc.tile_pool(name="scratch", bufs=4))
    outpool = ctx.enter_context(tc.tile_pool(name="outp", bufs=2))
    dram = ctx.enter_context(tc.tile_pool(name="dram", bufs=1, space="DRAM"))

    # ---- warm-up collective: absorbs CC startup + cross-core launch skew
    # so the real factor AllReduce later only pays the mesh transfer ----
    warm_in = dram.tile([1, 64], F32)
    warm_out = dram.tile([2, 64], F32)
    nc.gpsimd.collective_compute(
        "AllGather", ALU.bypass, replica_groups=RG,
        ins=[warm_in.opt()], outs=[warm_out.opt()],
    )
    warm_in2 = dram.tile([1, 64], F32)
    warm_out2 = dram.tile([2, 64], F32)

    # ---- one-time setup (small uploads on gpsimd; it idles afterwards so
    # the factor collective dispatches without queueing) ----
    wcat_sb = singles.tile([128, 2, PW], BF16)
    nc.sync.dma_start(wcat_sb[:], wcat.rearrange("(cb cp) w -> cp cb w", cb=2))
    bq_col = singles.tile([CQK, 1], F32)
    nc.gpsimd.dma_start(bq_col[:], bq_in)
    normc_rep = singles.tile([128, 2], F32)
    nc.gpsimd.dma_start(
        normc_rep[:], normc.unsqueeze(0).partition_broadcast(128).squeeze(1)
    )
    bv_rep = singles.tile([CQK, C], F32)
    nc.gpsimd.dma_start(
        bv_rep[:], bvg.unsqueeze(0).partition_broadcast(CQK).squeeze(1)
    )
    mxsel_sb = singles.tile([CQK + 2, CQK + 1], F32)
    nc.gpsimd.dma_start(mxsel_sb[:], mxsel)
    # rank-1 fold operands must sit at base partition 32 (matmul base
    # partitions are restricted to 0/32/64 and must match between operands)
    bkrow_sb = singles.tile([CQK + 1, CQK + 1], F32)   # row 32: [bk | 0]
    nc.gpsimd.dma_start(bkrow_sb[CQK : CQK + 1, :], mxrows[0:1, :])
    erow_sb = singles.tile([CQK + 1, CQK + 1], F32)    # row 32: [0..0 | 1]
    nc.gpsimd.dma_start(erow_sb[CQK : CQK + 1, :], mxrows[1:2, :])
    nbvrow_sb = singles.tile([CQK + 1, FD], F32)       # row 32: [N*g*bv | 0]
    nc.gpsimd.dma_start(nbvrow_sb[CQK : CQK + 1, :], nbvrow)
    ident = singles.tile([128, 128], F32)
    make_identity(nc, ident[:])

    # persistent rings: ones columns preset once
    kvring = singles.tile([128, 16, KVW], BF16)      # [Q | K | V | ones]
    nc.vector.memset(kvring[:, :, KVW - 1 : KVW], 1.0)
    khring = singles.tile([128, GRP, CQK + 2], BF16)  # [r*K | r | ones]
    nc.vector.memset(khring[:, :, CQK + 1 : CQK + 2], 1.0)

    qx = singles.tile([CQK + 1, NSH], BF16)          # c-major Q+bq rows + ||Q|| row
    ssq_stack = singles.tile([128, NT128], F32)      # sum(Qraw^2), col t
    ssk_stack = singles.tile([128, NT128], F32)      # sum(Kraw^2), col t
    crossqk = singles.tile([128, 2, NT128], F32)     # [2*bq.Qraw ; 2*bk.Kraw]
    cc_in = dram.tile([CQK + 2, FD], F32)
    cc_red = dram.tile([CQK + 2, FD], F32)

    xt_tiles = [None] * NT512

    with tc.tile_pool(name="ps_kqv", bufs=7, space="PSUM") as ps_kqv, tc.tile_pool(
        name="ps_f", bufs=1, space="PSUM"
    ) as ps_f:
        psf = ps_f.tile([CQK + 2, FD], F32)          # factor accumulator
        pending_tail = None

        def emit_tail(g0):
            nsum = smalls.tile([128, GRP], F32)
            nc.gpsimd.tensor_tensor(
                nsum[:], ssk_stack[:, g0 : g0 + GRP], crossqk[:, 1, g0 : g0 + GRP],
                ALU.add,
            )
            normk = smalls.tile([128, GRP], F32)
            nc.scalar.activation(
                normk[:], nsum[:], ACTF.Sqrt, bias=normc_rep[:, 1:2], scale=1.0
            )
            rnorm = smalls.tile([128, GRP], F32)
            nc.vector.reciprocal(rnorm[:], normk[:])
            # r column of the kh ring (slot tt%GRP), bf16 for the factor matmul
            nc.gpsimd.tensor_copy(khring[:, 0:GRP, CQK : CQK + 1], rnorm[:])
            for tt in range(g0, g0 + GRP):
                sl = tt % GRP
                nc.vector.tensor_scalar_mul(
                    khring[:, sl, 0:CQK],
                    kvring[:, tt % 16, CQK : 2 * CQK],
                    rnorm[:, sl : sl + 1],
                )
                nc.tensor.matmul(
                    psf[:], khring[:, sl, :], kvring[:, tt % 16, 2 * CQK : KVW],
                    start=(tt == 0), stop=(tt == NT128 - 1),
                )

        # ---- phase A: projections + norm accums + factor ----
        for j in range(NT512):
            xt = xpool.tile([128, 2, 512], BF16)
            nc.sync.dma_start(
                xt[:],
                xs.rearrange("(cb cp) n -> cp cb n", cb=2)[
                    :, :, j * 512 : (j + 1) * 512
                ],
            )
            xt_tiles[j] = xt

            for u in range(4):
                t = j * 4 + u
                if u == 2 and j % 2 == 0 and pending_tail is not None:
                    emit_tail(pending_tail)
                    pending_tail = None
                pskqv = ps_kqv.tile([128, PW], F32)
                for cb in range(2):
                    nc.tensor.matmul(
                        pskqv[:], xt[:, cb, u * 128 : (u + 1) * 128],
                        wcat_sb[:, cb, :],
                        start=(cb == 0), stop=(cb == 1),
                    )
                # Q|K|V -> ring (scalar, bf16 cast); squares accumulate from
                # the SBUF bf16 copy (engines read only one PSUM input)
                nc.scalar.copy(
                    kvring[:, t % 16, 0 : 2 * CQK + C], pskqv[:, 0 : 2 * CQK + C]
                )
                scr_q = scpool.tile([128, CQK], BF16)
                scr_k = scpool.tile([128, CQK], BF16)
                nc.vector.scalar_tensor_tensor(
                    scr_q[:], kvring[:, t % 16, 0:CQK], 1.0,
                    kvring[:, t % 16, 0:CQK],
                    ALU.mult, ALU.mult, accum_out=ssq_stack[:, t : t + 1],
                )
                nc.vector.scalar_tensor_tensor(
                    scr_k[:],
                    kvring[:, t % 16, CQK : 2 * CQK],
                    1.0,
                    kvring[:, t % 16, CQK : 2 * CQK],
                    ALU.mult, ALU.mult, accum_out=ssk_stack[:, t : t + 1],
                )
                nc.vector.tensor_copy(
                    crossqk[:, :, t : t + 1], pskqv[:, PW - 2 : PW]
                )

            if (j + 1) % (GRP // 4) == 0:
                pending_tail = (j + 1) * 4 - GRP
        if pending_tail is not None:
            emit_tail(pending_tail)
            pending_tail = None

        # ---- collective: fire as soon as the factor is done ----
        fac_loc = singles.tile([CQK + 2, FD], F32)
        nc.vector.tensor_copy(fac_loc[:], psf[:])
        nc.sync.dma_start(cc_in[:], fac_loc[:])
        nc.gpsimd.collective_compute(
            "AllReduce", ALU.add, replica_groups=RG,
            ins=[cc_in.opt()], outs=[cc_red.opt()],
        )

    with tc.tile_pool(name="ps_b", bufs=4, space="PSUM") as ps_b:
        # ---- phase B (hides the collective): c-major Q+bq, ||Q+bq|| row ----
        for j in range(NT512):
            psq = ps_b.tile([CQK, 512], F32, tag="shared")
            for cb in range(2):
                nc.tensor.matmul(
                    psq[:], wcat_sb[:, cb, 0:CQK], xt_tiles[j][:, cb, :],
                    start=(cb == 0), stop=(cb == 1),
                )
            if j % 8 < 5:
                nc.scalar.activation(
                    qx[0:CQK, j * 512 : (j + 1) * 512], psq[:],
                    ACTF.Identity, bias=bq_col[:], scale=1.0,
                )
            else:
                nc.vector.tensor_scalar_add(
                    qx[0:CQK, j * 512 : (j + 1) * 512], psq[:], bq_col[:]
                )

        ssq_tot = singles.tile([128, NT128], F32)
        nc.vector.tensor_tensor(ssq_tot[:], ssq_stack[:], crossqk[:, 0, :], ALU.add)
        normq_stack = singles.tile([128, NT128], F32)
        nc.scalar.activation(
            normq_stack[:], ssq_tot[:], ACTF.Sqrt, bias=normc_rep[:, 0:1], scale=1.0
        )
        pst = ps_b.tile([NT128, 128], F32, tag="shared")
        nc.tensor.transpose(pst[:], normq_stack[:], ident[:])
        trT = singles.tile([NT128, 128], BF16)
        nc.vector.tensor_copy(trT[:], pst[:])
        row_scratch = dram.tile([NT128, 128], BF16)
        nc.sync.dma_start(row_scratch[:], trT[:])
        nc.sync.dma_start(
            qx[CQK : CQK + 1, :],
            row_scratch[:].rearrange("a b -> (a b)").unsqueeze(0),
        )

        # ---- post-collective: facg + mx build (PE rank-1 folds) ----
        facg = singles.tile([CQK + 2, FD], F32)
        nc.sync.dma_start(facg[:], cc_red[:])
        ps_mx = ps_b.tile([CQK + 1, FD], F32, tag="shared")
        nc.tensor.matmul(ps_mx[:], mxsel_sb[:], facg[:], start=True, stop=False)
        nc.tensor.matmul(
            ps_mx[:],
            bkrow_sb[CQK : CQK + 1, :],
            facg[CQK : CQK + 1, :],
            start=False,
            stop=False,
        )
        nc.tensor.matmul(
            ps_mx[:],
            erow_sb[CQK : CQK + 1, :],
            nbvrow_sb[CQK : CQK + 1, :],
            start=False,
            stop=True,
        )
        mx = singles.tile([CQK + 1, FD], BF16)
        # denominator column first so phase 2's denominator matmuls can
        # start while the big fold below still runs
        nc.vector.tensor_scalar_add(
            mx[0:CQK, C : C + 1], ps_mx[0:CQK, C : C + 1], EPS
        )
        nc.vector.tensor_copy(mx[CQK : CQK + 1, :], ps_mx[CQK : CQK + 1, :])
        # rows 0:32: matrix + Ksum (x) bv'  (Ksum = ps_mx col 256)
        nc.vector.scalar_tensor_tensor(
            mx[0:CQK, 0:C], bv_rep[:], ps_mx[0:CQK, C : C + 1], ps_mx[0:CQK, 0:C],
            ALU.mult, ALU.add,
        )

        # ---- phase 2 ----
        with tc.tile_pool(name="ps_p2", bufs=3, space="PSUM") as ps_p2:
            out8 = out.rearrange("(g u p) c -> g p u c", u=GRP, p=128)
            for g8 in range(NT128 // GRP):
                ot = outpool.tile([128, GRP, C], BF16)
                for half in range(2):
                    g4 = g8 * 2 + half
                    psden = ps_p2.tile([128, 4], F32, tag="den", bufs=1)
                    for i in range(4):
                        t = g4 * 4 + i
                        nc.tensor.matmul(
                            psden[:, i : i + 1],
                            qx[:, t * 128 : (t + 1) * 128],
                            mx[:, C : C + 1],
                            start=True, stop=True,
                        )
                    rec4 = smalls.tile([128, 4], F32)
                    nc.vector.reciprocal(rec4[:], psden[:])
                    for i in range(4):
                        t = g4 * 4 + i
                        u = half * 4 + i
                        if i % 2 == 0:
                            ps2 = ps_b.tile([128, C], F32, tag="shared")
                        else:
                            ps2 = ps_p2.tile([128, C], F32, tag="p2")
                        nc.tensor.matmul(
                            ps2[:], qx[:, t * 128 : (t + 1) * 128], mx[:, 0:C],
                            start=True, stop=True,
                        )
                        if i % 2 == 0:
                            nc.vector.tensor_scalar_mul(
                                ot[:, u, :], ps2[:], rec4[:, i : i + 1]
                            )
                        else:
                            nc.scalar.mul(ot[:, u, :], ps2[:], rec4[:, i : i + 1])
                nc.sync.dma_start(out8[g8], ot[:])


def _get_nc():
    if "nc" not in _CACHE:
        _CACHE["nc"] = _build()
    return _CACHE["nc"]


def _prep_in_maps(x, Wq, bq, Wk, bk, Wv, bv, gamma):
    g = float(np.asarray(gamma).reshape(-1)[0])
    Wqf = np.asarray(Wq, np.float32)
    Wkf = np.asarray(Wk, np.float32)
    bqf = np.asarray(bq, np.float32)
    bkf = np.asarray(bk, np.float32)
    bvf = np.asarray(bv, np.float32)
    wcat = np.concatenate(
        [
            Wqf.T,
            Wkf.T,
            (g * np.asarray(Wv, np.float32)).T,
            (2.0 * Wqf.T @ bqf)[:, None],
            (2.0 * Wkf.T @ bkf)[:, None],
        ],
        axis=1,
    ).astype(ml_dtypes.bfloat16)
    wcat = np.ascontiguousarray(wcat)
    normc = np.array([bqf @ bqf, bkf @ bkf], np.float32)
    bvg = np.ascontiguousarray(g * bvf, dtype=np.float32)
    bq_col = np.ascontiguousarray(bqf.reshape(CQK, 1), dtype=np.float32)
    mxsel = np.zeros((CQK + 2, CQK + 1), np.float32)
    for i in range(CQK):
        mxsel[i, i] = 1.0
    mxsel[CQK + 1, CQK] = 1.0
    mxrows = np.zeros((2, CQK + 1), np.float32)
    mxrows[0, 0:CQK] = bkf
    mxrows[1, CQK] = 1.0
    nbvrow = np.zeros((1, FD), np.float32)
    nbvrow[0, 0:C] = float(N) * g * bvf

    xf = np.asarray(x, dtype=np.float32).reshape(B, C, N)
    in_maps = []
    for core in range(8):
        b, h = core // 2, core % 2
        xsh = np.ascontiguousarray(
            xf[b, :, h * NSH : (h + 1) * NSH].astype(ml_dtypes.bfloat16)
        )
        in_maps.append(
            {
                "xs": xsh,
                "wcat": wcat,
                "bq": bq_col,
                "normc": normc,
                "bvg": bvg,
                "mxsel": mxsel,
                "mxrows": mxrows,
                "nbvrow": nbvrow,
            }
        )
    return in_maps


def run(inputs, trace=False):
    nc = _get_nc()
    in_maps = _prep_in_maps(**inputs)
    res = bass_utils.run_bass_kernel_spmd(
        nc, in_maps, core_ids=list(range(8)), trace=trace
    )
    outf = np.empty((B, C, N), np.float32)
    for core in range(8):
        b, h = core // 2, core % 2
        outf[b, :, h * NSH : (h + 1) * NSH] = (
            res.results[core]["out"].astype(np.float32).T
        )
    return outf.reshape(B, C, HH, WW), res


def kernel(**inputs):
    out, _ = run(inputs, trace=False)
    return out


# revision 39
# speedup vs baseline: 1.3837x; 1.0155x over previous
"""Trainium2 Bass kernel for linear (taylor/sparse) attention.

Reference computation (per batch b, with xf = x.reshape(b, C, N)):
    Q = Wq@xf + bq            [Cqk, N]
    K = Wk@xf + bk            [Cqk, N]
    V = Wv@xf + bv            [C, N]
    Qh = Q / ||Q||_2 (per position, channel dim)
    Kh = K / ||K||_2
    tailor[n]   = 1 / (N + Qh[:,n] . (sum_n Kh + eps))
    matrix      = Kh @ V^T    [Cqk, C]
    out[:, n]   = gamma * tailor[n] * (sum_n V + matrix^T @ Qh[:,n])

Distribution: 8 cores = 4 batches x 2 halves of N (seq parallel), with a
pairwise AllReduce of the tiny factor.

v3 layout (per core, NSH=8192 positions = 64 tiles of 128):
  A dummy 256B collective fires at t=0 to absorb the CC engine's startup
  and the cross-core launch skew, so the real factor AllReduce later pays
  only the mesh transfer.
  Phase A per tile: fused projection [Qraw|Kraw|V'|2bq.Qraw|2bk.Kraw]
  (322 cols, V' = gamma*Wv@x) into a 6-deep PSUM rotation, Q|K|V copied to
  a persistent SBUF ring (scalar), sums-of-squares accumulated from the
  bf16 ring on DVE (bias-free norms: ||K+bk||^2 = ssk_raw + 2bk.Kraw +
  ||bk||^2, cross terms as extra projection columns), and per 8-tile group
  the K-norm reciprocals scale the kh ring so the factor
      psf[34, 257] = [r*Kraw | r | 1]^T @ [V' | 1]
  accumulates bias-free (bk folds in post-collective as a rank-1 PE
  update: mx = sel^T.facg + bk (x) facg_row32 + N*bv' row).  GpSimd stays
  nearly idle in phase A so the AllReduce dispatches the moment the factor
  is ready; phase B (c-major Q+bq via matmul with per-partition bias, the
  ||Q+bq|| row via sqrt + PE transpose) hides the collective latency.
  Phase 2 per 4 tiles: denominator-column matmuls into a shared PSUM bank
  + one batched reciprocal, then [128, 256] matmuls into a 7-deep PSUM
  rotation, scaled on alternating Scalar/Vector into bf16, DMA'd out 8
  tiles per descriptor.
Output is n-major bf16 [N_shard, C]; the host transposes/casts back.
"""

import ml_dtypes
import numpy as np
from contextlib import ExitStack

import concourse.bass as bass
import concourse.bacc as bacc
import concourse.tile as tile
from concourse import mybir
from concourse import bass_utils
from concourse.masks import make_identity

F32 = mybir.dt.float32
BF16 = mybir.dt.bfloat16
ALU = mybir.AluOpType
ACTF = mybir.ActivationFunctionType

B, C, HH, WW = 4, 256, 128, 128
N = HH * WW            # 16384 positions per batch
NSH = N // 2           # 8192 positions per core
CQK = 32
PW = 2 * CQK + C + 2   # 322: [Q | K | V | qcross | kcross] fused projection
KVW = 2 * CQK + C + 2  # 322: kvring slot = [Q | K | V | ones | pad]
                       # (even width keeps every slot 4B-aligned for DVE 2x)
FD = C + 1             # 257: factor free width ([V | 1])
NT512 = NSH // 512     # 16
NT128 = NSH // 128     # 64
GRP = 8                # tiles per K-norm batching group
EPS = 1e-6
RG = [[0, 1], [2, 3], [4, 5], [6, 7]]

_CACHE = {}


def _build():
    nc = bacc.Bacc("TRN2", target_bir_lowering=False, debug=False, num_devices=8)

    xs = nc.dram_tensor("xs", [C, NSH], BF16, kind="ExternalInput").ap()
    wcat = nc.dram_tensor("wcat", [C, PW], BF16, kind="ExternalInput").ap()
    bq_in = nc.dram_tensor("bq", [CQK, 1], F32, kind="ExternalInput").ap()
    normc = nc.dram_tensor("normc", [2], F32, kind="ExternalInput").ap()
    bvg = nc.dram_tensor("bvg", [C], F32, kind="ExternalInput").ap()
    mxsel = nc.dram_tensor("mxsel", [CQK + 2, CQK + 1], F32, kind="ExternalInput").ap()
    mxrows = nc.dram_tensor("mxrows", [2, CQK + 1], F32, kind="ExternalInput").ap()
    nbvrow = nc.dram_tensor("nbvrow", [1, FD], F32, kind="ExternalInput").ap()
    out = nc.dram_tensor("out", [NSH, C], BF16, kind="ExternalOutput").ap()

    with tile.TileContext(nc) as tc, ExitStack() as ctx:
        _body(ctx, tc, nc, xs, wcat, bq_in, normc, bvg, mxsel, mxrows, nbvrow, out)

    nc.compile()
    return nc


def _body(ctx, tc, nc, xs, wcat, bq_in, normc, bvg, mxsel, mxrows, nbvrow, out):
    singles = ctx.enter_context(tc.tile_pool(name="singles", bufs=1))
    xpool = ctx.enter_context(tc.tile_pool(name="x", bufs=NT512))
    smalls = ctx.enter_context(tc.tile_pool(name="smalls", bufs=4))
    scpool = ctx.enter_context(tc.tile_pool(name="scratch", bufs=4))
    outpool = ctx.enter_context(tc.tile_pool(name="outp", bufs=2))
    dram = ctx.enter_context(tc.tile_pool(name="dram", bufs=1, space="DRAM"))

    # ---- warm-up collective: absorbs CC startup + cross-core launch skew
    # so the real factor AllReduce later only pays the mesh transfer ----
    warm_in = dram.tile([1, 64], F32)
    warm_out = dram.tile([2, 64], F32)
    nc.gpsimd.collective_compute(
        "AllGather", ALU.bypass, replica_groups=RG,
        ins=[warm_in.opt()], outs=[warm_out.opt()],
    )
    warm_in2 = dram.tile([1, 64], F32)
    warm_out2 = dram.tile([2, 64], F32)

    # ---- one-time setup (small uploads on gpsimd; it idles afterwards so
    # the factor collective dispatches without queueing) ----
    wcat_sb = singles.tile([128, 2, PW], BF16)
    nc.sync.dma_start(wcat_sb[:], wcat.rearrange("(cb cp) w -> cp cb w", cb=2))
    bq_col = singles.tile([CQK, 1], F32)
    nc.gpsimd.dma_start(bq_col[:], bq_in)
    normc_rep = singles.tile([128, 2], F32)
    nc.gpsimd.dma_start(
        normc_rep[:], normc.unsqueeze(0).partition_broadcast(128).squeeze(1)
    )
    bv_rep = singles.tile([CQK, C], F32)
    nc.gpsimd.dma_start(
        bv_rep[:], bvg.unsqueeze(0).partition_broadcast(CQK).squeeze(1)
    )
    mxsel_sb = singles.tile([CQK + 2, CQK + 1], F32)
    nc.gpsimd.dma_start(mxsel_sb[:], mxsel)
    # rank-1 fold operands must sit at base partition 32 (matmul base
    # partitions are restricted to 0/32/64 and must match between operands)
    bkrow_sb = singles.tile([CQK + 1, CQK + 1], F32)   # row 32: [bk | 0]
    nc.gpsimd.dma_start(bkrow_sb[CQK : CQK + 1, :], mxrows[0:1, :])
    erow_sb = singles.tile([CQK + 1, CQK + 1], F32)    # row 32: [0..0 | 1]
    nc.gpsimd.dma_start(erow_sb[CQK : CQK + 1, :], mxrows[1:2, :])
    nbvrow_sb = singles.tile([CQK + 1, FD], F32)       # row 32: [N*g*bv | 0]
    nc.gpsimd.dma_start(nbvrow_sb[CQK : CQK + 1, :], nbvrow)
    ident = singles.tile([128, 128], F32)
    make_identity(nc, ident[:])

    # persistent rings: ones columns preset once
    kvring = singles.tile([128, 16, KVW], BF16)      # [Q | K | V | ones | pad]
    nc.vector.memset(kvring[:, :, KVW - 2 : KVW - 1], 1.0)
    khring = singles.tile([128, GRP, CQK + 2], BF16)  # [r*K | r | ones]
    nc.vector.memset(khring[:, :, CQK + 1 : CQK + 2], 1.0)

    qx = singles.tile([CQK + 1, NSH], BF16)          # c-major Q+bq rows + ||Q|| row
    ssq_stack = singles.tile([128, NT128], F32)      # sum(Qraw^2), col t
    ssk_stack = singles.tile([128, NT128], F32)      # sum(Kraw^2), col t
    crossqk = singles.tile([128, 2, NT128], F32)     # [2*bq.Qraw ; 2*bk.Kraw]
    cc_in = dram.tile([CQK + 2, FD], F32)
    cc_red = dram.tile([CQK + 2, FD], F32)

    xt_tiles = [None] * NT512

    with tc.tile_pool(name="ps_kqv", bufs=7, space="PSUM") as ps_kqv, tc.tile_pool(
        name="ps_f", bufs=1, space="PSUM"
    ) as ps_f:
        psf = ps_f.tile([CQK + 2, FD], F32)          # factor accumulator
        pending_tail = None

        def emit_tail(g0):
            nsum = smalls.tile([128, GRP], F32)
            nc.gpsimd.tensor_tensor(
                nsum[:], ssk_stack[:, g0 : g0 + GRP], crossqk[:, 1, g0 : g0 + GRP],
                ALU.add,
            )
            normk = smalls.tile([128, GRP], F32)
            nc.scalar.activation(
                normk[:], nsum[:], ACTF.Sqrt, bias=normc_rep[:, 1:2], scale=1.0
            )
            rnorm = smalls.tile([128, GRP], F32)
            nc.vector.reciprocal(rnorm[:], normk[:])
            # r column of the kh ring (slot tt%GRP), bf16 for the factor matmul
            nc.gpsimd.tensor_copy(khring[:, 0:GRP, CQK : CQK + 1], rnorm[:])
            for tt in range(g0, g0 + GRP):
                sl = tt % GRP
                nc.vector.tensor_scalar_mul(
                    khring[:, sl, 0:CQK],
                    kvring[:, tt % 16, CQK : 2 * CQK],
                    rnorm[:, sl : sl + 1],
                )
                nc.tensor.matmul(
                    psf[:], khring[:, sl, :], kvring[:, tt % 16, 2 * CQK : KVW - 1],
                    start=(tt == 0), stop=(tt == NT128 - 1),
                )

        # ---- phase A: projections + norm accums + factor ----
        for j in range(NT512):
            xt = xpool.tile([128, 2, 512], BF16)
            nc.sync.dma_start(
                xt[:],
                xs.rearrange("(cb cp) n -> cp cb n", cb=2)[
                    :, :, j * 512 : (j + 1) * 512
                ],
            )
            xt_tiles[j] = xt

            for u in range(4):
                t = j * 4 + u
                if u == 2 and j % 2 == 0 and pending_tail is not None:
                    emit_tail(pending_tail)
                    pending_tail = None
                pskqv = ps_kqv.tile([128, PW], F32)
                for cb in range(2):
                    nc.tensor.matmul(
                        pskqv[:], xt[:, cb, u * 128 : (u + 1) * 128],
                        wcat_sb[:, cb, :],
                        start=(cb == 0), stop=(cb == 1),
                    )
                # Q|K|V -> ring (scalar, bf16 cast); squares accumulate from
                # the SBUF bf16 copy (engines read only one PSUM input)
                nc.scalar.copy(
                    kvring[:, t % 16, 0 : 2 * CQK + C], pskqv[:, 0 : 2 * CQK + C]
                )
                scr_q = scpool.tile([128, CQK], BF16)
                scr_k = scpool.tile([128, CQK], BF16)
                nc.vector.scalar_tensor_tensor(
                    scr_q[:], kvring[:, t % 16, 0:CQK], 1.0,
                    kvring[:, t % 16, 0:CQK],
                    ALU.mult, ALU.mult, accum_out=ssq_stack[:, t : t + 1],
                )
                nc.vector.scalar_tensor_tensor(
                    scr_k[:],
                    kvring[:, t % 16, CQK : 2 * CQK],
                    1.0,
                    kvring[:, t % 16, CQK : 2 * CQK],
                    ALU.mult, ALU.mult, accum_out=ssk_stack[:, t : t + 1],
                )
                nc.vector.tensor_copy(
                    crossqk[:, :, t : t + 1], pskqv[:, PW - 2 : PW]
                )

            if (j + 1) % (GRP // 4) == 0:
                pending_tail = (j + 1) * 4 - GRP
        if pending_tail is not None:
            emit_tail(pending_tail)
            pending_tail = None

        # ---- collective: fire as soon as the factor is done ----
        fac_loc = singles.tile([CQK + 2, FD], F32)
        nc.vector.tensor_copy(fac_loc[:], psf[:])
        nc.sync.dma_start(cc_in[:], fac_loc[:])
        nc.gpsimd.collective_compute(
            "AllReduce", ALU.add, replica_groups=RG,
            ins=[cc_in.opt()], outs=[cc_red.opt()],
        )

    with tc.tile_pool(name="ps_b", bufs=4, space="PSUM") as ps_b:
        # ---- phase B (hides the collective): c-major Q+bq, ||Q+bq|| row ----
        for j in range(NT512):
            psq = ps_b.tile([CQK, 512], F32, tag="shared")
            for cb in range(2):
                nc.tensor.matmul(
                    psq[:], wcat_sb[:, cb, 0:CQK], xt_tiles[j][:, cb, :],
                    start=(cb == 0), stop=(cb == 1),
                )
            if j % 2 == 0:
                nc.scalar.activation(
                    qx[0:CQK, j * 512 : (j + 1) * 512], psq[:],
                    ACTF.Identity, bias=bq_col[:], scale=1.0,
                )
            else:
                nc.vector.tensor_scalar_add(
                    qx[0:CQK, j * 512 : (j + 1) * 512], psq[:], bq_col[:]
                )

        ssq_tot = singles.tile([128, NT128], F32)
        nc.vector.tensor_tensor(ssq_tot[:], ssq_stack[:], crossqk[:, 0, :], ALU.add)
        normq_stack = singles.tile([128, NT128], F32)
        nc.scalar.activation(
            normq_stack[:], ssq_tot[:], ACTF.Sqrt, bias=normc_rep[:, 0:1], scale=1.0
        )
        pst = ps_b.tile([NT128, 128], F32, tag="shared")
        nc.tensor.transpose(pst[:], normq_stack[:], ident[:])
        trT = singles.tile([NT128, 128], BF16)
        nc.vector.tensor_copy(trT[:], pst[:])
        row_scratch = dram.tile([NT128, 128], BF16)
        nc.sync.dma_start(row_scratch[:], trT[:])
        nc.sync.dma_start(
            qx[CQK : CQK + 1, :],
            row_scratch[:].rearrange("a b -> (a b)").unsqueeze(0),
        )

        # ---- post-collective: facg + mx build (PE rank-1 folds) ----
        facg = singles.tile([CQK + 2, FD], F32)
        nc.sync.dma_start(facg[:], cc_red[:])
        ps_mx = ps_b.tile([CQK + 1, FD], F32, tag="shared")
        nc.tensor.matmul(ps_mx[:], mxsel_sb[:], facg[:], start=True, stop=False)
        nc.tensor.matmul(
            ps_mx[:],
            bkrow_sb[CQK : CQK + 1, :],
            facg[CQK : CQK + 1, :],
            start=False,
            stop=False,
        )
        nc.tensor.matmul(
            ps_mx[:],
            erow_sb[CQK : CQK + 1, :],
            nbvrow_sb[CQK : CQK + 1, :],
            start=False,
            stop=True,
        )
        mx = singles.tile([CQK + 1, FD], BF16)
        # denominator column first so phase 2's denominator matmuls can
        # start while the big fold below still runs
        nc.vector.tensor_scalar_add(
            mx[0:CQK, C : C + 1], ps_mx[0:CQK, C : C + 1], EPS
        )
        nc.vector.tensor_copy(mx[CQK : CQK + 1, :], ps_mx[CQK : CQK + 1, :])
        # rows 0:32: matrix + Ksum (x) bv'  (Ksum = ps_mx col 256)
        nc.vector.scalar_tensor_tensor(
            mx[0:CQK, 0:C], bv_rep[:], ps_mx[0:CQK, C : C + 1], ps_mx[0:CQK, 0:C],
            ALU.mult, ALU.add,
        )

        # ---- phase 2 ----
        with tc.tile_pool(name="ps_p2", bufs=3, space="PSUM") as ps_p2:
            out8 = out.rearrange("(g u p) c -> g p u c", u=GRP, p=128)
            for g8 in range(NT128 // GRP):
                ot = outpool.tile([128, GRP, C], BF16)
                for half in range(2):
                    g4 = g8 * 2 + half
                    # each qx stationary loaded once: denominator column +
                    # numerator matmul back-to-back, scales trail the
                    # batched reciprocal
                    psden = ps_p2.tile([128, 4], F32, tag="den", bufs=1)
                    ps2s = []
                    for i in range(4):
                        t = g4 * 4 + i
                        nc.tensor.matmul(
                            psden[:, i : i + 1],
                            qx[:, t * 128 : (t + 1) * 128],
                            mx[:, C : C + 1],
                            start=True, stop=True,
                        )
                        if i % 2 == 0:
                            ps2 = ps_b.tile([128, C], F32, tag="shared")
                        else:
                            ps2 = ps_p2.tile([128, C], F32, tag="p2")
                        ps2s.append(ps2)
                        nc.tensor.matmul(
                            ps2[:], qx[:, t * 128 : (t + 1) * 128], mx[:, 0:C],
                            start=True, stop=True,
                        )
                    rec4 = smalls.tile([128, 4], F32)
                    nc.vector.reciprocal(rec4[:], psden[:])
                    for i in range(4):
                        u = half * 4 + i
                        if i % 2 == 0:
                            nc.vector.tensor_scalar_mul(
                                ot[:, u, :], ps2s[i][:], rec4[:, i : i + 1]
                            )
                        else:
                            nc.scalar.mul(ot[:, u, :], ps2s[i][:], rec4[:, i : i + 1])
                nc.sync.dma_start(out8[g8], ot[:])


def _get_nc():
    if "nc" not in _CACHE:
        _CACHE["nc"] = _build()
    return _CACHE["nc"]


def _prep_in_maps(x, Wq, bq, Wk, bk, Wv, bv, gamma):
    g = float(np.asarray(gamma).reshape(-1)[0])
    Wqf = np.asarray(Wq, np.float32)
    Wkf = np.asarray(Wk, np.float32)
    bqf = np.asarray(bq, np.float32)
    bkf = np.asarray(bk, np.float32)
    bvf = np.asarray(bv, np.float32)
    wcat = np.concatenate(
        [
            Wqf.T,
            Wkf.T,
            (g * np.asarray(Wv, np.float32)).T,
            (2.0 * Wqf.T @ bqf)[:, None],
            (2.0 * Wkf.T @ bkf)[:, None],
        ],
        axis=1,
    ).astype(ml_dtypes.bfloat16)
    wcat = np.ascontiguousarray(wcat)
    normc = np.array([bqf @ bqf, bkf @ bkf], np.float32)
    bvg = np.ascontiguousarray(g * bvf, dtype=np.float32)
    bq_col = np.ascontiguousarray(bqf.reshape(CQK, 1), dtype=np.float32)
    mxsel = np.zeros((CQK + 2, CQK + 1), np.float32)
    for i in range(CQK):
        mxsel[i, i] = 1.0
    mxsel[CQK + 1, CQK] = 1.0
    mxrows = np.zeros((2, CQK + 1), np.float32)
    mxrows[0, 0:CQK] = bkf
    mxrows[1, CQK] = 1.0
    nbvrow = np.zeros((1, FD), np.float32)
    nbvrow[0, 0:C] = float(N) * g * bvf

    xf = np.asarray(x, dtype=np.float32).reshape(B, C, N)
    in_maps = []
    for core in range(8):
        b, h = core // 2, core % 2
        xsh = np.ascontiguousarray(
            xf[b, :, h * NSH : (h + 1) * NSH].astype(ml_dtypes.bfloat16)
        )
        in_maps.append(
            {
                "xs": xsh,
                "wcat": wcat,
                "bq": bq_col,
                "normc": normc,
                "bvg": bvg,
                "mxsel": mxsel,
                "mxrows": mxrows,
                "nbvrow": nbvrow,
            }
        )
    return in_maps


def run(inputs, trace=False):
    nc = _get_nc()
    in_maps = _prep_in_maps(**inputs)
    res = bass_utils.run_bass_kernel_spmd(
        nc, in_maps, core_ids=list(range(8)), trace=trace
    )
    outf = np.empty((B, C, N), np.float32)
    for core in range(8):
        b, h = core // 2, core % 2
        outf[b, :, h * NSH : (h + 1) * NSH] = (
            res.results[core]["out"].astype(np.float32).T
        )
    return outf.reshape(B, C, HH, WW), res


def kernel(**inputs):
    out, _ = run(inputs, trace=False)
    return out
